# revision 54
# baseline (speedup 1.0000x reference)
"""Causal self-attention (B=1, S=4096, D=768, H=12) on 8 TRN2 NeuronCores.

Sharding: 4 head-groups (3 heads each) x 2 query-parity halves; no
collectives. Core c = 2*g + p handles heads [3g, 3g+3) and query rows
{r : r % 2 == p} (strided assignment balances causal work perfectly).

V4 highlights (on top of V3's dual-streamed scores, causal band
truncation, pack-interleaved exp, and filler-threaded projections):
  - All inputs arrive host-pre-swizzled into exact SBUF tile layouts so
    every input DMA is a dense [128, N] full-rate copy, issued on ONE
    queue in strict consumption order (HBM bw is shared across queues).
  - m=1 (64-row) K/Q projection halves run as col-tiled PAIRS (two M=64
    matmuls in one [128,512] psum, array col groups 0-1 / 2-3), halving
    their PE time; same trick dual-streams pairs of K=64 wo1 matmuls
    via aT2/wo1 duplicated into both partition halves.
  - Fillers are budgeted per tile (12t+8 pumps) with just-in-time
    deadlines so no projection ever dumps into a serialized ping-pong;
    6-chunk projections split across two pumps where packs are scarce.
  - PE preheat + keep-warm dummy matmuls hold the HAM clock gate at
    8/8 (2.4 GHz) through the ramp, spare pumps, and the epilogue.
  - The last 4 row-blocks' out-projection is split: the wo0 part runs
    as late fillers inside tile 3's phase B, the wo1 part + identity
    re-injection of the staged wo0 + CAST + DMA form a short epilogue
    with evacuations/DMAs alternated across engines/queues.

All matmuls run in bf16 (f32 PSUM accumulation); softmax exp in f32.
"""
import os

import numpy as np
import ml_dtypes

import concourse.bass as bass
import concourse.mybir as mybir
import concourse.tile as tile
from concourse import bacc
from concourse.bass_utils import run_bass_kernel_spmd

BF16 = mybir.dt.bfloat16
F32 = mybir.dt.float32
NPBF16 = ml_dtypes.bfloat16

S = 4096          # sequence length
D = 768           # model dim
HD = 64           # head dim
HL = 3            # heads per core
DL = HL * HD      # 192 local qkv cols per core
SQ = S // 2       # 2048 local queries per core
NQT = 4           # q-tiles per core
QTW = 512         # q-tile width (local queries)
NKB = S // 128    # 32 key blocks of 128
NDC = D // 128    # 6 contraction chunks of 128 over D
VW = HD + 1       # V' column stride per head (64 V cols + ones col)
SCALE = HD ** -0.5

# band packs: diagonal blocks b paired so each pack's widths sum to <=512
BAND_PACKS = ((0,), (1, 7), (2, 6), (3, 5), (4,))

# divide-by-softmax-sum via a partition-broadcast DVE read instead of a
# PE broadcast matmul (saves ~12 matmuls + a DVE op per divide)
DIVIDE_BCAST = False  # zero-step partition APs rejected on DVE and DMA paths


def build_nc():
    # xT / xqT / wk / wq / wv arrive pre-swizzled from the host into the
    # exact SBUF tile layouts, so every input DMA is a dense [128, N] copy
    # (full-rate contiguous segments, no strided descriptor storms).
    nc = bacc.Bacc(None, target_bir_lowering=False)
    xT = nc.declare_dram_parameter("xT", [128, 8 * NDC * 512], BF16,
                                   isOutput=False)
    xqT = nc.declare_dram_parameter("xqT", [128, 4 * NDC * 512], BF16,
                                    isOutput=False)
    wk = nc.declare_dram_parameter("wk", [128, NDC * DL], BF16, isOutput=False)
    wq = nc.declare_dram_parameter("wq", [128, NDC * DL], BF16, isOutput=False)
    wv = nc.declare_dram_parameter("wv", [128, NDC * DL], BF16, isOutput=False)
    bkq = nc.declare_dram_parameter("bkq", [DL, 2], F32, isOutput=False)
    bv = nc.declare_dram_parameter("bv", [DL], F32, isOutput=False)
    wout = nc.declare_dram_parameter("wout", [DL, D], BF16, isOutput=False)
    mask64 = nc.declare_dram_parameter("mask64", [128, 64], BF16, isOutput=False)
    ident = nc.declare_dram_parameter("ident", [128, 128], BF16, isOutput=False)
    out = nc.declare_dram_parameter("out", [SQ, D], BF16, isOutput=True)

    from contextlib import ExitStack

    with tile.TileContext(nc) as tc, ExitStack() as ctx:
        persist = ctx.enter_context(tc.tile_pool(name="persist", bufs=1))
        xtp = ctx.enter_context(tc.tile_pool(name="xtp", bufs=1))
        wp = ctx.enter_context(tc.tile_pool(name="wp", bufs=1))
        o0p = ctx.enter_context(tc.tile_pool(name="o0p", bufs=1))
        pjp = ctx.enter_context(tc.tile_pool(name="pjp", bufs=2, space="PSUM"))
        psp = ctx.enter_context(tc.tile_pool(name="psp", bufs=1, space="PSUM"))
        pop = ctx.enter_context(tc.tile_pool(name="pop", bufs=2, space="PSUM"))
        ep = ctx.enter_context(tc.tile_pool(name="ep", bufs=2))
        rp = ctx.enter_context(tc.tile_pool(name="rp", bufs=2))
        osb = ctx.enter_context(tc.tile_pool(name="osb", bufs=3))

        kT01 = persist.tile([128, S], BF16)         # K^T heads 0,1
        kT2 = persist.tile([128, S], BF16)          # K^T head 2 (both halves)
        qT01 = persist.tile([128, SQ], BF16)        # Q^T heads 0,1
        qT2 = persist.tile([128, SQ], BF16)         # Q^T head 2 (both halves)
        aT01 = persist.tile([128, SQ], BF16)        # attn^T heads 0,1
        aT2 = persist.tile([128, SQ], BF16)         # head 2, dup halves
        vbig = persist.tile([128, NKB * HL * VW], BF16)  # V' blocks [k,195]
        bvb = persist.tile([128, DL], F32)          # bv broadcast over rows
        msk = persist.tile([128, 64], BF16)         # causal triangle r<=2c+p
        identp = persist.tile([128, 128], BF16)     # identity (epilogue)
        ones1 = persist.tile([1, 64], BF16)
        bkq0 = persist.tile([128, 2], F32)
        bkq1 = persist.tile([128, 2], F32)          # [0:64] and [64:128] same
        wo0 = persist.tile([128, D], BF16)
        wo1 = persist.tile([128, D], BF16)  # wout[128:DL] duplicated in
        # both partition halves so two wo1 matmuls can dual-stream the PE

        warm = persist.tile([128, 64], BF16)
        nc.gpsimd.memset(warm, 0.125)
        # only V's softmax-sum columns (col 64 of each head's VW stride)
        # need the 1.0 fill; v_proj overwrites the rest. A strided memset
        # keeps the gpsimd queue free for the bias/mask DMAs the head needs.
        nc.gpsimd.memset(
            vbig.rearrange("p (k h v) -> p k h v", h=HL, v=VW)[:, :, :, HD:VW],
            1.0)
        nc.gpsimd.memset(ones1, 1.0)

        # x^T / xq^T land as 512-column slices holding all 6 contraction
        # chunks: tile cols = kc*512 + j. Weights land as [128, 6*DL].
        xt = [xtp.tile([128, NDC * 512], BF16, name=f"xt{n}") for n in range(8)]
        xq = [xtp.tile([128, NDC * 512], BF16, name=f"xq{t}") for t in range(NQT)]
        wk_t = wp.tile([128, NDC * DL], BF16, name="wk")
        wq_t = wp.tile([128, NDC * DL], BF16, name="wq")
        wv_t = wp.tile([128, NDC * DL], BF16, name="wv")

        # input DMAs: dense [128, N] copies (host pre-swizzled), spread
        # across the 3 DMA-capable queues, need-ordered within each queue.
        # xt[0]/xq[0] split by contraction chunk so the head's first
        # matmuls start as soon as their chunk lands.
        TW = NDC * 512  # 3072 cols per x tile

        def dma_x(xts, n, eng, c0=0, c1=NDC):
            eng.dma_start(out=xts[n][:, c0 * 512:c1 * 512],
                          in_=(xT if xts is xt else xqT)[
                              :, n * TW + c0 * 512:n * TW + c1 * 512])

        # Strict need-priority on ONE queue: HBM bandwidth (~360GB/s/core)
        # is shared across queues, so concurrent queues starve the
        # early-critical transfers. Everything big goes on sync in exact
        # consumption order; gpsimd carries only the small tensors.
        nc.sync.dma_start(out=wk_t, in_=wk[:, :])
        dma_x(xt, 0, nc.sync, 0, 2)
        dma_x(xt, 0, nc.sync, 2, 4)
        dma_x(xt, 0, nc.sync, 4, 6)
        nc.sync.dma_start(out=wv_t, in_=wv[:, :])
        nc.sync.dma_start(out=wq_t, in_=wq[:, :])
        dma_x(xq, 0, nc.sync, 0, 3)
        dma_x(xq, 0, nc.sync, 3, 6)
        dma_x(xt, 1, nc.sync)
        dma_x(xq, 1, nc.sync)
        dma_x(xt, 2, nc.sync)
        dma_x(xt, 3, nc.sync)
        dma_x(xt, 4, nc.sync)
        dma_x(xq, 2, nc.sync)
        dma_x(xq, 3, nc.sync)
        dma_x(xt, 5, nc.sync)
        dma_x(xt, 6, nc.sync)
        dma_x(xt, 7, nc.sync)

        # small tensors on the gpsimd queue
        nc.gpsimd.dma_start(out=bkq0, in_=bkq[0:128, :])
        nc.gpsimd.dma_start(out=bkq1[0:64, :], in_=bkq[128:DL, :])
        nc.gpsimd.dma_start(out=bkq1[64:128, :], in_=bkq[128:DL, :])
        nc.gpsimd.dma_start(out=bvb, in_=bv[:].partition_broadcast(128))
        nc.gpsimd.dma_start(out=msk, in_=mask64[:, :])
        nc.gpsimd.dma_start(out=wo0, in_=wout[0:128, :])
        nc.gpsimd.dma_start(out=wo1[0:64, :], in_=wout[128:DL, :])
        nc.gpsimd.dma_start(out=wo1[64:128, :], in_=wout[128:DL, :])
        nc.gpsimd.dma_start(out=identp, in_=ident[:, :])

        # PE preheat: tiny matmuls during the initial DMA wait so the HAM
        # clock-gate's busy window starts counting before the real head
        # projections begin (flip to 8/8 needs ~3.4us of sustained PE
        # activity).
        pwarm = pjp.tile([128, 512], F32, name="pwarm", tag="pj")
        for _ in range(12):
            nc.tensor.matmul(pwarm[0:64, 0:64], lhsT=warm, rhs=warm,
                             start=True, stop=True, skip_group_check=True)

        def kq_proj(dst01, dst2, w_t, rhs, bc, n, m, head=False):
            # dst[m-rows, cols n*512..] = W^T x^T + b  for one m-pass.
            # head=True: rotate through the (idle) score PSUM slots and
            # evacuate on the (idle) scalar engine - no single-slot WAR
            # stall, no DVE backlog.
            nsl = slice(n * 512, (n + 1) * 512)
            mw = 128 if m == 0 else 64
            msl = slice(0, 128) if m == 0 else slice(128, DL)
            if head:
                ps = psp.tile([128, 1024], F32, name="ps", tag="ps",
                              bufs=2)[:, 0:512]
            else:
                ps = pjp.tile([128, 512], F32, name="pj", tag="pj")
            for kc in range(NDC):
                nc.tensor.matmul(
                    ps[:mw, :],
                    lhsT=w_t[:, kc * DL:(kc + 1) * DL][:, msl],
                    rhs=rhs[:, kc * 512:(kc + 1) * 512],
                    start=(kc == 0), stop=(kc == NDC - 1),
                )
            ident = mybir.ActivationFunctionType.Identity

            def evac(dst, src, bias):
                if head:
                    nc.scalar.activation(out=dst, in_=src, func=ident,
                                         bias=bias)
                else:
                    nc.vector.tensor_scalar_add(out=dst, in0=src,
                                                scalar1=bias)

            if m == 0:
                evac(dst01[:, nsl], ps, bkq0[:, bc:bc + 1])
            else:  # head 2: write both partition halves (dual-tile scores)
                evac(dst2[0:64, nsl], ps[:64, :], bkq1[0:64, bc:bc + 1])
                evac(dst2[64:128, nsl], ps[:64, :], bkq1[64:128, bc:bc + 1])

        kqp_state = {}

        def kq1_pair(specs, head=False, part=None):
            # Two m=1 (64-row) projections col-tiled side by side: chain a
            # writes psum partitions 0:64 (array col groups 0-1), chain b
            # partitions 64:128 (groups 2-3); the PE streams both
            # concurrently, halving the m=1 projection time. part=0/1
            # optionally spreads the chunks over two filler pumps.
            key = tuple(s[4] for s in specs)
            if part == 1:
                ps = kqp_state.pop(key)
            elif head:
                ps = psp.tile([128, 1024], F32, name="ps", tag="ps",
                              bufs=2)[:, 0:512]
            else:
                ps = pjp.tile([128, 512], F32, name="pj", tag="pj")
            if part == 0:
                kqp_state[key] = ps
            chunks = (range(0, 3) if part == 0
                      else range(3, NDC) if part == 1 else range(NDC))
            for kc in chunks:
                for half, (dst2, w_t, rhs, bc, n) in zip((0, 64), specs):
                    nc.tensor.matmul(
                        ps[half:half + 64, :],
                        lhsT=w_t[:, kc * DL:(kc + 1) * DL][:, 128:DL],
                        rhs=rhs[:, kc * 512:(kc + 1) * 512],
                        start=(kc == 0), stop=(kc == NDC - 1),
                        skip_group_check=True,
                    )
            if part == 0:
                return
            identf = mybir.ActivationFunctionType.Identity
            for half, (dst2, w_t, rhs, bc, n) in zip((0, 64), specs):
                nsl = slice(n * 512, (n + 1) * 512)
                for dhalf in (0, 64):
                    src = ps[half:half + 64, :]
                    dst = dst2[dhalf:dhalf + 64, nsl]
                    bias = bkq1[dhalf:dhalf + 64, bc:bc + 1]
                    if head:
                        nc.scalar.activation(out=dst, in_=src, func=identf,
                                             bias=bias)
                    else:
                        nc.vector.tensor_scalar_add(out=dst, in0=src,
                                                    scalar1=bias)

        def v_proj(kb, head=False):
            if head:
                pv = psp.tile([128, 1024], F32, name="ps", tag="ps",
                              bufs=2)[:, 0:512]
            else:
                pv = pjp.tile([128, 512], F32, name="pj", tag="pj")
            n, j = kb // 4, (kb % 4) * 128
            for kc in range(NDC):
                nc.tensor.matmul(
                    pv[:, :DL], lhsT=xt[n][:, kc * 512 + j:kc * 512 + j + 128],
                    rhs=wv_t[:, kc * DL:(kc + 1) * DL],
                    start=(kc == 0), stop=(kc == NDC - 1),
                )
            # one strided add writes all 3 heads' V cols (ones col skipped)
            voff = kb * HL * VW
            dstv = vbig[:, voff:voff + HL * VW]
            dstv = dstv.rearrange("p (h vw) -> p h vw", vw=VW)[:, :, 0:HD]
            nc.vector.tensor_add(
                out=dstv,
                in0=pv[:, :DL].rearrange("p (h d) -> p h d", d=HD),
                in1=bvb.rearrange("p (h d) -> p h d", d=HD),
            )

        # per-head score/attn tiles: (lhsT source, rhs source, aT dest)
        kq_src = (
            (kT01, qT01, (0, 64)),     # head 0: always low half
            (kT01, qT01, (64, 128)),   # head 1: always high half
            (kT2, qT2, None),          # head 2: half chosen per matmul
        )
        aT_of = (aT01[0:64], aT01[64:128], aT2[0:64])

        ot_tiles = {}

        def out_proj_half(qt, ncol, pot=None, reg=0, act_copy=False):
            # split at the PSUM-slot reuse boundary so the WAR wait on the
            # previous half's copy never stalls the PE mid-filler
            osl = slice(qt * 128, (qt + 1) * 128)
            if ncol == 0:
                ot_tiles[qt] = osb.tile([128, D], BF16, name="ot", tag="ot")
            ot = ot_tiles[qt]
            cw = 512 if ncol == 0 else 256
            csl = slice(ncol * 512, ncol * 512 + cw)
            final = pot is not None
            if not final:
                pot = pjp.tile([128, 512], F32, name="pj", tag="pj")
            psl = slice(reg * 512, reg * 512 + cw)
            nc.tensor.matmul(
                pot[:, psl], lhsT=aT01[:, osl], rhs=wo0[:, csl],
                start=True, stop=False, skip_group_check=True)
            nc.tensor.matmul(
                pot[:, psl], lhsT=aT2[0:64, osl], rhs=wo1[0:64, csl],
                start=False, stop=True, skip_group_check=True)
            if act_copy:  # tail: split evacuations across DVE and ACT
                nc.scalar.activation(out=ot[:, csl], in_=pot[:, psl],
                                     func=mybir.ActivationFunctionType.Copy)
            else:
                nc.vector.tensor_copy(out=ot[:, csl], in_=pot[:, psl])
            if final:
                nc.gpsimd.dma_start(out=out[osl, csl], in_=ot[:, csl])
            elif ncol == 1:
                nc.gpsimd.dma_start(out=out[osl, :], in_=ot)

        def attention(t, fillers, late_fillers=(), late_start=0):
            late_fillers = list(late_fillers)
            nseen = [0]

            def pump(k=1):
                for _ in range(k):
                    nseen[0] += 1
                    if fillers:
                        fillers.pop(0)()
                    elif late_fillers and nseen[0] > late_start:
                        late_fillers.pop(0)()
                    elif k == 1 and nseen[0] % 2 == 0:
                        # spare pump: tiny keep-warm matmul so partial PE
                        # idle never accumulates into a HAM MID window
                        # (re-throttle to 1.2 GHz costs ~2us per event)
                        nc.tensor.matmul(pwarm[0:64, 0:64], lhsT=warm,
                                         rhs=warm, start=True, stop=True,
                                         skip_group_check=True)

            qoff = t * QTW
            last_kb = 8 * t + BAND_PACKS[-1][-1]

            # entry = (half, head, kb, psum_off, width, q_start, band)
            # pack = (entries, exp_ranges, heads_finishing)
            packs = []
            # phase A: heads 0+1 paired on alternating PE row tiles
            for kb in range(0, 8 * t):
                packs.append((
                    [(0, 0, kb, 0, 512, 0, False),
                     (64, 1, kb, 512, 512, 0, False)],
                    ((0, 1024),), ()))
            for pr in BAND_PACKS[:-1]:
                ent = []
                offs = [0, 512]
                for b in pr:
                    w = 512 - 64 * b
                    for h in (0, 1):
                        ent.append((64 * h, h, 8 * t + b, offs[h], w,
                                    64 * b, True))
                        offs[h] += w
                packs.append((ent, ((0, 1024),), ()))
            packs.append((
                [(0, 0, 8 * t + 4, 0, 256, 256, True),
                 (64, 1, 8 * t + 4, 512, 256, 256, True)],
                ((0, 256), (512, 768)), (0, 1)))
            # phase B: head 2 alternating its two duplicated halves
            for kb in range(0, 8 * t, 2):
                packs.append((
                    [(0, 2, kb, 0, 512, 0, False),
                     (64, 2, kb + 1, 512, 512, 0, False)],
                    ((0, 1024),), ()))
            for i, prs in enumerate((((0,), (1, 7)), ((2, 6), (3, 5)))):
                ent = []
                for j, pr in enumerate(prs):
                    off = 512 * j
                    for b in pr:
                        w = 512 - 64 * b
                        ent.append((64 * ((2 * i + j) % 2), 2, 8 * t + b,
                                    off, w, 64 * b, True))
                        off += w
                packs.append((ent, ((0, 1024),), ()))
            packs.append((
                [(0, 2, 8 * t + 4, 0, 256, 256, True)],
                ((0, 256),), (2,)))

            po_of = {}

            def emit_pv(pack, eT):
                entries, _, fin = pack
                for (_half, h, kb, off, w, qs, _band) in entries:
                    if h not in po_of:
                        # lazy: the slot's previous reader (divide of the
                        # evicted head) must already be emitted for the WAR
                        po_of[h] = pop.tile([VW, 512], F32, name="po",
                                            tag="po")
                    voff = kb * HL * VW + h * VW
                    nc.tensor.matmul(
                        po_of[h][0:VW, qs:qs + w],
                        lhsT=vbig[:, voff:voff + VW],
                        rhs=eT[:, off:off + w],
                        start=(kb == 0), stop=(kb == last_kb),
                        skip_group_check=True,
                    )
                for h in fin:
                    divide(h)

            def divide(h):
                # divide by the softmax sum (row HD of po). The 1->64
                # partition broadcast of the reciprocal goes over a gpsimd
                # DMA (off the PE); the final tile-3 head-2 divide keeps
                # the PE broadcast matmul since it sits on the critical
                # tail and the PE is idle there anyway.
                po = po_of[h]
                if DIVIDE_BCAST and not (t == 3 and h == 2):
                    rec1 = rp.tile([1, 512], F32, name="rec1", tag="rec1")
                    nc.vector.reciprocal_approx_fast(out=rec1,
                                                     in_=po[HD:VW, :])
                    recb = rp.tile([64, 512], F32, name="recb", tag="recb")
                    nc.gpsimd.dma_start(out=recb,
                                        in_=rec1.partition_broadcast(64))
                else:
                    sums = rp.tile([1, 512], BF16, name="sums", tag="sums")
                    nc.vector.tensor_copy(out=sums, in_=po[HD:VW, :])
                    pb = pjp.tile([128, 512], F32, name="pj", tag="pj")
                    nc.tensor.matmul(pb[0:64, :], lhsT=ones1, rhs=sums,
                                     start=True, stop=True)
                    recb = rp.tile([64, 512], F32, name="recb", tag="recb")
                    nc.vector.reciprocal_approx_fast(out=recb,
                                                     in_=pb[0:64, :])
                nc.vector.tensor_mul(
                    out=aT_of[h][:, qoff:qoff + QTW], in0=po[0:HD, :],
                    in1=recb)
                if h == 2:  # mirror into the upper half for dual-stream
                    nc.vector.tensor_copy(
                        out=aT2[64:128, qoff:qoff + QTW],
                        in_=aT2[0:64, qoff:qoff + QTW])

            pend = None  # (pack, eT) whose PV is not yet emitted
            for pack in packs:
                entries, exp_ranges, _fin = pack
                ps = psp.tile([128, 1024], F32, name="ps", tag="ps", bufs=2)
                for (half, h, kb, off, w, qs, _band) in entries:
                    kT_h, qT_h, fixed = kq_src[h]
                    if fixed is not None:
                        hsl = slice(fixed[0], fixed[1])
                    else:
                        hsl = slice(half, half + 64)
                    nc.tensor.matmul(
                        ps[:, off:off + w],
                        lhsT=kT_h[hsl, kb * 128:(kb + 1) * 128],
                        rhs=qT_h[hsl, qoff + qs:qoff + QTW],
                        start=True, stop=True,
                    )
                eT = ep.tile([128, 1024], BF16, name="eT", tag="eT", bufs=3)
                for (r0, r1) in exp_ranges:
                    nc.scalar.activation(
                        out=eT[:, r0:r1], in_=ps[:, r0:r1],
                        func=mybir.ActivationFunctionType.Exp, scale=SCALE)
                for (_half, _h, kb, off, w, qs, band) in entries:
                    if band:  # zero the 64 partial cols of the triangle
                        nc.vector.tensor_mul(
                            out=eT[:, off:off + 64],
                            in0=eT[:, off:off + 64], in1=msk)
                pump(1)
                if pend is not None:
                    emit_pv(*pend)
                pend = (pack, eT)
            emit_pv(*pend)
            pump(len(fillers) + len(late_fillers))

        # wo0-part of the final 4 row-blocks' out-projection: runs as late
        # fillers inside tile 3's phase B (aT01 for t=3 is ready once heads
        # 0/1 finish at the end of phase A); wo1 + combine run in the
        # epilogue once aT2 lands.
        ot0 = {qt: o0p.tile([128, D], BF16, name=f"ot0_{qt}")
               for qt in (12, 13, 14, 15)}

        def out_proj_pair(qta, qtb, ncol):
            # two row-blocks' out-projection halves: the wo0 parts run
            # back-to-back, then the two K=64 wo1 parts dual-stream via
            # aT2's duplicated partition halves.
            cw = 512 if ncol == 0 else 256
            csl = slice(ncol * 512, ncol * 512 + cw)
            pots = []
            for qt in (qta, qtb):
                osl = slice(qt * 128, (qt + 1) * 128)
                if ncol == 0:
                    ot_tiles[qt] = osb.tile([128, D], BF16, name="ot",
                                            tag="ot")
                pot = pjp.tile([128, 512], F32, name="pj", tag="pj")
                nc.tensor.matmul(
                    pot[:, :cw], lhsT=aT01[:, osl], rhs=wo0[:, csl],
                    start=True, stop=False, skip_group_check=True)
                pots.append(pot)
            for i, qt in enumerate((qta, qtb)):
                osl = slice(qt * 128, (qt + 1) * 128)
                hsl = slice(64 * i, 64 * i + 64)
                nc.tensor.matmul(
                    pots[i][:, :cw], lhsT=aT2[hsl, osl], rhs=wo1[hsl, csl],
                    start=False, stop=True, skip_group_check=True)
            for i, qt in enumerate((qta, qtb)):
                osl = slice(qt * 128, (qt + 1) * 128)
                ot = ot_tiles[qt]
                nc.vector.tensor_copy(out=ot[:, csl], in_=pots[i][:, :cw])
                if ncol == 1:
                    nc.gpsimd.dma_start(out=out[osl, :], in_=ot)

        kqh_state = {}

        def kq_half(key, dst01, dst2, w_t, rhs, bc, n, m, part):
            # half of a 6-chunk projection: spreads one filler across two
            # pump slots so filler-heavy packs don't outrun the exp pace
            if part == 0:
                ps = pjp.tile([128, 512], F32, name="pj", tag="pj")
                kqh_state[key] = ps
            else:
                ps = kqh_state.pop(key)
            mw = 128 if m == 0 else 64
            msl = slice(0, 128) if m == 0 else slice(128, DL)
            for kc in (range(0, 3) if part == 0 else range(3, NDC)):
                nc.tensor.matmul(
                    ps[:mw, :],
                    lhsT=w_t[:, kc * DL:(kc + 1) * DL][:, msl],
                    rhs=rhs[:, kc * 512:(kc + 1) * 512],
                    start=(kc == 0), stop=(kc == NDC - 1),
                )
            if part == 1:
                nsl = slice(n * 512, (n + 1) * 512)
                nc.vector.tensor_scalar_add(out=dst01[:, nsl], in0=ps,
                                            scalar1=bkq0[:, bc:bc + 1])

        def wo0_half(qt, ncol):
            osl = slice(qt * 128, (qt + 1) * 128)
            cw = 512 if ncol == 0 else 256
            csl = slice(ncol * 512, ncol * 512 + cw)
            pw = pjp.tile([128, 512], F32, name="pj", tag="pj")
            nc.tensor.matmul(pw[:, :cw], lhsT=aT01[:, osl], rhs=wo0[:, csl],
                             start=True, stop=True, skip_group_check=True)
            nc.vector.tensor_copy(out=ot0[qt][:, csl], in_=pw[:, :cw])

        # ---- schedule: minimal head, then q-tiles t=0..3 with fillers ----
        def K(n, m):
            return lambda: kq_proj(kT01, kT2, wk_t, xt[n], 0, n, m)

        def Q(t, m):
            return lambda: kq_proj(qT01, qT2, wq_t, xq[t], 1, t, m)

        def V(kb):
            return lambda: v_proj(kb)

        def O(qt, ncol):
            return lambda: out_proj_half(qt, ncol)

        def W0(qt, ncol):
            return lambda: wo0_half(qt, ncol)

        def O2(qta, qtb, ncol):
            return lambda: out_proj_pair(qta, qtb, ncol)

        def KH(n, part):
            return lambda: kq_half(("k", n), kT01, kT2, wk_t, xt[n], 0,
                                   n, 0, part)

        def QH(t, part):
            return lambda: kq_half(("q", t), qT01, qT2, wq_t, xq[t], 1,
                                   t, 0, part)

        def KP1(na, nb, part=None):
            return lambda: kq1_pair([(kT2, wk_t, xt[na], 0, na),
                                     (kT2, wk_t, xt[nb], 0, nb)], part=part)

        def QP1(ta, tb):
            return lambda: kq1_pair([(qT2, wq_t, xq[ta], 1, ta),
                                     (qT2, wq_t, xq[tb], 1, tb)])

        # head: only what attention(0)'s phase A needs, DMA-ordered.
        # m=1 halves run as col-tiled pairs (same wall time as one); the
        # q-side m=1 pair rides tile-0's fillers (deadline: its phase B),
        # so the head never waits on the xq1 transfer.
        def hwarm(k=4):
            # bridge the head's DMA-wait pockets with dummy matmuls so the
            # HAM activity window keeps counting and the attention stream
            # enters already at 2.4 GHz
            for _ in range(k):
                nc.tensor.matmul(pwarm[0:64, 0:64], lhsT=warm, rhs=warm,
                                 start=True, stop=True,
                                 skip_group_check=True)

        kq_proj(kT01, kT2, wk_t, xt[0], 0, 0, 0, head=True)
        hwarm()
        v_proj(0, head=True)
        v_proj(1, head=True)
        hwarm()
        kq_proj(qT01, qT2, wq_t, xq[0], 1, 0, 0, head=True)
        hwarm()
        kq_proj(kT01, kT2, wk_t, xt[1], 0, 1, 0, head=True)
        kq1_pair([(kT2, wk_t, xt[0], 0, 0), (kT2, wk_t, xt[1], 0, 1)],
                 head=True)
        v_proj(7, head=True)

        # V fillers ordered by the band-block usage order of the next tile
        def Vband(t):
            return [V(8 * t + b) for b in (0, 1, 7, 2, 6, 3, 5, 4)]

        # Fillers sized to each tile's pack count (12t+8 pumps) so no tile
        # ends with a serialized dump; each tile's prerequisites (K/Q/V of
        # the NEXT tile) sit early enough in the list to land just in time.
        FILL = {
            0: [V(2), V(6), QP1(0, 1), V(3), V(5), V(4), Q(1, 0), K(2, 0)],
            1: [KH(3, 0), KH(3, 1), KP1(2, 3)] + Vband(1) +
               [KH(4, 0), KH(4, 1), KH(5, 0), KH(5, 1), KP1(4, 5),
                QH(2, 0), QH(2, 1), QP1(2, 3)],
            2: Vband(2) + [KH(6, 0), KH(6, 1), KH(7, 0), KH(7, 1),
                           KP1(6, 7, 0), KP1(6, 7, 1), QH(3, 0), QH(3, 1)],
            3: Vband(3) + [O2(0, 1, 0), O2(0, 1, 1), O2(2, 3, 0),
                           O2(2, 3, 1), O2(4, 5, 0), O2(4, 5, 1),
                           O2(6, 7, 0), O2(6, 7, 1), O2(8, 9, 0),
                           O2(8, 9, 1), O2(10, 11, 0), O2(10, 11, 1)],
        }
        LATE3 = [W0(12, 0), W0(12, 1), W0(13, 0), W0(13, 1),
                 W0(14, 0), W0(14, 1), W0(15, 0), W0(15, 1)]
        for t in range(NQT):
            if t == 3:
                attention(t, FILL[t], late_fillers=LATE3, late_start=8 * t + 6)
            else:
                attention(t, FILL[t])
        # epilogue: wo1-part of the final 4 row-blocks (needs aT2 of tile
        # 3). The staged wo0-part is re-injected into PSUM via an identity
        # matmul (PE is idle here), the wo1 pairs dual-stream via aT2's
        # duplicated halves, and a cheap 2x CAST copy replaces a DVE add.
        # Keep-warm dummies bridge the divide(2) stall so the epilogue
        # matmuls run at 2.4 GHz.
        for _ in range(12):
            nc.tensor.matmul(pwarm[0:64, 0:64], lhsT=warm, rhs=warm,
                             start=True, stop=True, skip_group_check=True)
        otf_of = {}
        for (qta, ncola), (qtb, ncolb) in (((12, 0), (13, 0)),
                                           ((14, 0), (15, 0)),
                                           ((12, 1), (13, 1)),
                                           ((14, 1), (15, 1))):
            pt = psp.tile([128, 1024], F32, name="ps", tag="ps", bufs=2)
            for i, (qt, ncol) in enumerate(((qta, ncola), (qtb, ncolb))):
                osl = slice(qt * 128, (qt + 1) * 128)
                cw = 512 if ncol == 0 else 256
                csl = slice(ncol * 512, ncol * 512 + cw)
                nc.tensor.matmul(
                    pt[:, 512 * i:512 * i + cw], lhsT=identp,
                    rhs=ot0[qt][:, csl], start=True, stop=False,
                    skip_group_check=True)
            for i, (qt, ncol) in enumerate(((qta, ncola), (qtb, ncolb))):
                osl = slice(qt * 128, (qt + 1) * 128)
                hsl = slice(64 * i, 64 * i + 64)
                cw = 512 if ncol == 0 else 256
                csl = slice(ncol * 512, ncol * 512 + cw)
                nc.tensor.matmul(
                    pt[:, 512 * i:512 * i + cw], lhsT=aT2[hsl, osl],
                    rhs=wo1[hsl, csl], start=False, stop=True,
                    skip_group_check=True)
            for i, (qt, ncol) in enumerate(((qta, ncola), (qtb, ncolb))):
                osl = slice(qt * 128, (qt + 1) * 128)
                cw = 512 if ncol == 0 else 256
                csl = slice(ncol * 512, ncol * 512 + cw)
                if qt not in otf_of:
                    otf_of[qt] = osb.tile([128, D], BF16, name="ot", tag="ot")
                otf = otf_of[qt]
                # evacuations and output DMAs alternate engines/queues so
                # the two per-pair chains drain concurrently
                if i == 0:
                    nc.scalar.activation(
                        out=otf[:, csl], in_=pt[:, 0:cw],
                        func=mybir.ActivationFunctionType.Copy)
                    nc.sync.dma_start(out=out[osl, csl], in_=otf[:, csl])
                else:
                    nc.vector.tensor_copy(out=otf[:, csl],
                                          in_=pt[:, 512:512 + cw])
                    nc.gpsimd.dma_start(out=out[osl, csl], in_=otf[:, csl])

    nc.finalize()
    return nc


_NC_CACHE = {}


def _get_nc():
    if "nc" not in _NC_CACHE:
        _NC_CACHE["nc"] = build_nc()
    return _NC_CACHE["nc"]


def kernel(x, Wqkv, bqkv, Wout, bout):
    x = np.asarray(x, dtype=np.float32)
    Wqkv = np.asarray(Wqkv, dtype=np.float32)
    bqkv = np.asarray(bqkv, dtype=np.float32)
    Wout = np.asarray(Wout, dtype=np.float32)
    bout = np.asarray(bout, dtype=np.float32)
    B, S_, D_ = x.shape
    assert (B, S_, D_) == (1, S, D)
    nc = _get_nc()

    xT_np = np.ascontiguousarray(x[0].T).astype(NPBF16)          # [768, 4096]

    def swizzle_x(a):
        # [768, ncols] -> [128, ntiles*6*512]: SBUF tile layout (tile-major,
        # then contraction chunk, then 512 cols) so device DMAs are dense.
        ncols = a.shape[1]
        nt = ncols // 512
        return np.ascontiguousarray(
            a.reshape(6, 128, nt, 512).transpose(1, 2, 0, 3).reshape(
                128, nt * 6 * 512))

    def swizzle_w(w):
        # [768, DL] -> [128, 6*DL]
        return np.ascontiguousarray(
            w.reshape(6, 128, DL).transpose(1, 0, 2).reshape(128, 6 * DL))

    xT_sw = swizzle_x(xT_np)
    xq_sw = [swizzle_x(np.ascontiguousarray(xT_np[:, p::2])) for p in (0, 1)]
    in_maps = []
    for c in range(8):
        g, p = c // 2, c % 2
        csl = slice(DL * g, DL * (g + 1))
        rr = np.arange(128, dtype=np.int64)[:, None]
        cc = np.arange(64, dtype=np.int64)[None, :]
        mask = (rr <= 2 * cc + p).astype(NPBF16)
        bk_h = bqkv[D + DL * g:D + DL * (g + 1)].astype(np.float32)
        bq_h = bqkv[csl].astype(np.float32)
        in_maps.append({
            "xT": xT_sw,
            "xqT": xq_sw[p],
            "wk": swizzle_w(Wqkv[:, D + DL * g:D + DL * (g + 1)].astype(NPBF16)),
            "wq": swizzle_w(Wqkv[:, csl].astype(NPBF16)),
            "wv": swizzle_w(Wqkv[:, 2 * D + DL * g:2 * D + DL * (g + 1)].astype(NPBF16)),
            "bkq": np.ascontiguousarray(np.stack([bk_h, bq_h], axis=1)),
            "bv": np.ascontiguousarray(bqkv[2 * D + DL * g:2 * D + DL * (g + 1)]).astype(np.float32),
            "wout": np.ascontiguousarray(Wout[csl, :]).astype(NPBF16),
            "mask64": mask,
            "ident": np.eye(128, dtype=NPBF16),
        })

    trace = bool(int(os.environ.get("ATTN_TRACE", "0")))
    tmpdir = os.environ.get("ATTN_TMPDIR") or None
    res = run_bass_kernel_spmd(nc, in_maps, core_ids=list(range(8)), trace=trace,
                               tmpdir=tmpdir)
    if trace:
        _NC_CACHE["last_result"] = res

    out_full = np.zeros((S, D), np.float32)
    for p in range(2):
        acc = np.zeros((SQ, D), np.float32)
        for g in range(4):
            acc += res.results[2 * g + p]["out"].astype(np.float32)
        out_full[p::2] = acc
    out_full += bout.astype(np.float32)[None, :]
    return out_full[None].astype(np.float32)



# revision 55
# speedup vs baseline: 1.1899x; 1.1899x over previous
"""Causal self-attention (B=1, S=4096, D=768, H=12) on 8 TRN2 NeuronCores.

Sharding: 4 head-groups (3 heads each) x 2 query-parity halves; no
collectives. Core c = 2*g + p handles heads [3g, 3g+3) and query rows
{r : r % 2 == p} (strided assignment balances causal work perfectly).

V4 highlights (on top of V3's dual-streamed scores, causal band
truncation, pack-interleaved exp, and filler-threaded projections):
  - All inputs arrive host-pre-swizzled into exact SBUF tile layouts so
    every input DMA is a dense [128, N] full-rate copy, issued on ONE
    queue in strict consumption order (HBM bw is shared across queues).
  - m=1 (64-row) K/Q projection halves run as col-tiled PAIRS (two M=64
    matmuls in one [128,512] psum, array col groups 0-1 / 2-3), halving
    their PE time; same trick dual-streams pairs of K=64 wo1 matmuls
    via aT2/wo1 duplicated into both partition halves.
  - Fillers are budgeted per tile (12t+8 pumps) with just-in-time
    deadlines so no projection ever dumps into a serialized ping-pong;
    6-chunk projections split across two pumps where packs are scarce.
  - PE preheat + keep-warm dummy matmuls hold the HAM clock gate at
    8/8 (2.4 GHz) through the ramp, spare pumps, and the epilogue.
  - The last 4 row-blocks' out-projection is split: the wo0 part runs
    as late fillers inside tile 3's phase B, the wo1 part + identity
    re-injection of the staged wo0 + CAST + DMA form a short epilogue
    with evacuations/DMAs alternated across engines/queues.

All matmuls run in bf16 (f32 PSUM accumulation); softmax exp in f32.
"""
import os

import numpy as np
import ml_dtypes

import concourse.bass as bass
import concourse.mybir as mybir
import concourse.tile as tile
from concourse import bacc
from concourse.bass_utils import run_bass_kernel_spmd

BF16 = mybir.dt.bfloat16
F32 = mybir.dt.float32
NPBF16 = ml_dtypes.bfloat16

S = 4096          # sequence length
D = 768           # model dim
HD = 64           # head dim
HL = 3            # heads per core
DL = HL * HD      # 192 local qkv cols per core
SQ = S // 2       # 2048 local queries per core
NQT = 4           # q-tiles per core
QTW = 512         # q-tile width (local queries)
NKB = S // 128    # 32 key blocks of 128
NDC = D // 128    # 6 contraction chunks of 128 over D
VW = HD + 1       # V' column stride per head (64 V cols + ones col)
SCALE = HD ** -0.5

# band packs: diagonal blocks b paired so each pack's widths sum to <=512
BAND_PACKS = ((0,), (1, 7), (2, 6), (3, 5), (4,))

# divide-by-softmax-sum via a partition-broadcast DVE read instead of a
# PE broadcast matmul (saves ~12 matmuls + a DVE op per divide)
DIVIDE_BCAST = False  # zero-step partition APs rejected on DVE and DMA paths


def build_nc():
    # xT / xqT / wk / wq / wv arrive pre-swizzled from the host into the
    # exact SBUF tile layouts, so every input DMA is a dense [128, N] copy
    # (full-rate contiguous segments, no strided descriptor storms).
    nc = bacc.Bacc(None, target_bir_lowering=False)
    xT = nc.declare_dram_parameter("xT", [128, 8 * NDC * 512], BF16,
                                   isOutput=False)
    xqT = nc.declare_dram_parameter("xqT", [128, 4 * NDC * 512], BF16,
                                    isOutput=False)
    wk = nc.declare_dram_parameter("wk", [128, NDC * DL], BF16, isOutput=False)
    wq = nc.declare_dram_parameter("wq", [128, NDC * DL], BF16, isOutput=False)
    wv = nc.declare_dram_parameter("wv", [128, NDC * DL], BF16, isOutput=False)
    bkq = nc.declare_dram_parameter("bkq", [DL, 2], F32, isOutput=False)
    bv = nc.declare_dram_parameter("bv", [DL], F32, isOutput=False)
    wout = nc.declare_dram_parameter("wout", [DL, D], BF16, isOutput=False)
    mask64 = nc.declare_dram_parameter("mask64", [128, 64], BF16, isOutput=False)
    ident = nc.declare_dram_parameter("ident", [128, 128], BF16, isOutput=False)
    out = nc.declare_dram_parameter("out", [SQ, D], BF16, isOutput=True)

    from contextlib import ExitStack

    with tile.TileContext(nc) as tc, ExitStack() as ctx:
        persist = ctx.enter_context(tc.tile_pool(name="persist", bufs=1))
        xtp = ctx.enter_context(tc.tile_pool(name="xtp", bufs=1))
        wp = ctx.enter_context(tc.tile_pool(name="wp", bufs=1))
        o0p = ctx.enter_context(tc.tile_pool(name="o0p", bufs=1))
        pjp = ctx.enter_context(tc.tile_pool(name="pjp", bufs=2, space="PSUM"))
        psp = ctx.enter_context(tc.tile_pool(name="psp", bufs=1, space="PSUM"))
        pop = ctx.enter_context(tc.tile_pool(name="pop", bufs=2, space="PSUM"))
        ep = ctx.enter_context(tc.tile_pool(name="ep", bufs=2))
        rp = ctx.enter_context(tc.tile_pool(name="rp", bufs=2))
        osb = ctx.enter_context(tc.tile_pool(name="osb", bufs=3))

        kT01 = persist.tile([128, S], BF16)         # K^T heads 0,1
        kT2 = persist.tile([128, S], BF16)          # K^T head 2 (both halves)
        qT01 = persist.tile([128, SQ], BF16)        # Q^T heads 0,1
        qT2 = persist.tile([128, SQ], BF16)         # Q^T head 2 (both halves)
        aT01 = persist.tile([128, SQ], BF16)        # attn^T heads 0,1
        aT2 = persist.tile([128, SQ], BF16)         # head 2, dup halves
        vbig = persist.tile([128, NKB * HL * VW], BF16)  # V' blocks [k,195]
        bvb = persist.tile([128, DL], F32)          # bv broadcast over rows
        msk = persist.tile([128, 64], BF16)         # causal triangle r<=2c+p
        identp = persist.tile([128, 128], BF16)     # identity (epilogue)
        ones1 = persist.tile([1, 64], BF16)
        bkq0 = persist.tile([128, 2], F32)
        bkq1 = persist.tile([128, 2], F32)          # [0:64] and [64:128] same
        wo0 = persist.tile([128, D], BF16)
        wo1 = persist.tile([128, D], BF16)  # wout[128:DL] duplicated in
        # both partition halves so two wo1 matmuls can dual-stream the PE

        warm = persist.tile([128, 64], BF16)
        nc.gpsimd.memset(warm, 0.125)
        # only V's softmax-sum columns (col 64 of each head's VW stride)
        # need the 1.0 fill; v_proj overwrites the rest. A strided memset
        # keeps the gpsimd queue free for the bias/mask DMAs the head needs.
        nc.gpsimd.memset(
            vbig.rearrange("p (k h v) -> p k h v", h=HL, v=VW)[:, :, :, HD:VW],
            1.0)
        nc.gpsimd.memset(ones1, 1.0)

        # x^T / xq^T land as 512-column slices holding all 6 contraction
        # chunks: tile cols = kc*512 + j. Weights land as [128, 6*DL].
        xt = [xtp.tile([128, NDC * 512], BF16, name=f"xt{n}") for n in range(8)]
        xq = [xtp.tile([128, NDC * 512], BF16, name=f"xq{t}") for t in range(NQT)]
        wk_t = wp.tile([128, NDC * DL], BF16, name="wk")
        wq_t = wp.tile([128, NDC * DL], BF16, name="wq")
        wv_t = wp.tile([128, NDC * DL], BF16, name="wv")

        # input DMAs: dense [128, N] copies (host pre-swizzled), spread
        # across the 3 DMA-capable queues, need-ordered within each queue.
        # xt[0]/xq[0] split by contraction chunk so the head's first
        # matmuls start as soon as their chunk lands.
        TW = NDC * 512  # 3072 cols per x tile

        def dma_x(xts, n, eng, c0=0, c1=NDC):
            eng.dma_start(out=xts[n][:, c0 * 512:c1 * 512],
                          in_=(xT if xts is xt else xqT)[
                              :, n * TW + c0 * 512:n * TW + c1 * 512])

        # Strict need-priority on ONE queue: HBM bandwidth (~360GB/s/core)
        # is shared across queues, so concurrent queues starve the
        # early-critical transfers. Everything big goes on sync in exact
        # consumption order; gpsimd carries only the small tensors.
        nc.sync.dma_start(out=wk_t, in_=wk[:, :])
        dma_x(xt, 0, nc.sync, 0, 2)
        dma_x(xt, 0, nc.sync, 2, 4)
        dma_x(xt, 0, nc.sync, 4, 6)
        nc.sync.dma_start(out=wv_t, in_=wv[:, :])
        nc.sync.dma_start(out=wq_t, in_=wq[:, :])
        dma_x(xq, 0, nc.sync, 0, 3)
        dma_x(xq, 0, nc.sync, 3, 6)
        dma_x(xt, 1, nc.sync)
        dma_x(xq, 1, nc.sync)
        dma_x(xt, 2, nc.sync)
        dma_x(xt, 3, nc.sync)
        dma_x(xt, 4, nc.sync)
        dma_x(xq, 2, nc.sync)
        dma_x(xq, 3, nc.sync)
        dma_x(xt, 5, nc.sync)
        dma_x(xt, 6, nc.sync)
        dma_x(xt, 7, nc.sync)

        # small tensors on the gpsimd queue
        nc.gpsimd.dma_start(out=bkq0, in_=bkq[0:128, :])
        nc.gpsimd.dma_start(out=bkq1[0:64, :], in_=bkq[128:DL, :])
        nc.gpsimd.dma_start(out=bkq1[64:128, :], in_=bkq[128:DL, :])
        nc.gpsimd.dma_start(out=bvb, in_=bv[:].partition_broadcast(128))
        nc.gpsimd.dma_start(out=msk, in_=mask64[:, :])
        nc.gpsimd.dma_start(out=wo0, in_=wout[0:128, :])
        nc.gpsimd.dma_start(out=wo1[0:64, :], in_=wout[128:DL, :])
        nc.gpsimd.dma_start(out=wo1[64:128, :], in_=wout[128:DL, :])
        nc.gpsimd.dma_start(out=identp, in_=ident[:, :])

        # PE preheat: tiny matmuls during the initial DMA wait so the HAM
        # clock-gate's busy window starts counting before the real head
        # projections begin (flip to 8/8 needs ~3.4us of sustained PE
        # activity).
        pwarm = pjp.tile([128, 512], F32, name="pwarm", tag="pj")
        for _ in range(12):
            nc.tensor.matmul(pwarm[0:64, 0:64], lhsT=warm, rhs=warm,
                             start=True, stop=True, skip_group_check=True)

        def kq_proj(dst01, dst2, w_t, rhs, bc, n, m, head=False):
            # dst[m-rows, cols n*512..] = W^T x^T + b  for one m-pass.
            # head=True: rotate through the (idle) score PSUM slots and
            # evacuate on the (idle) scalar engine - no single-slot WAR
            # stall, no DVE backlog.
            nsl = slice(n * 512, (n + 1) * 512)
            mw = 128 if m == 0 else 64
            msl = slice(0, 128) if m == 0 else slice(128, DL)
            if head:
                ps = psp.tile([128, 1024], F32, name="ps", tag="ps",
                              bufs=2)[:, 0:512]
            else:
                ps = pjp.tile([128, 512], F32, name="pj", tag="pj")
            for kc in range(NDC):
                nc.tensor.matmul(
                    ps[:mw, :],
                    lhsT=w_t[:, kc * DL:(kc + 1) * DL][:, msl],
                    rhs=rhs[:, kc * 512:(kc + 1) * 512],
                    start=(kc == 0), stop=(kc == NDC - 1),
                )
            ident = mybir.ActivationFunctionType.Identity

            def evac(dst, src, bias):
                if head:
                    nc.scalar.activation(out=dst, in_=src, func=ident,
                                         bias=bias)
                else:
                    nc.vector.tensor_scalar_add(out=dst, in0=src,
                                                scalar1=bias)

            if m == 0:
                evac(dst01[:, nsl], ps, bkq0[:, bc:bc + 1])
            else:  # head 2: write both partition halves (dual-tile scores)
                evac(dst2[0:64, nsl], ps[:64, :], bkq1[0:64, bc:bc + 1])
                evac(dst2[64:128, nsl], ps[:64, :], bkq1[64:128, bc:bc + 1])

        kqp_state = {}

        def kq1_pair(specs, head=False, part=None):
            # Two m=1 (64-row) projections col-tiled side by side: chain a
            # writes psum partitions 0:64 (array col groups 0-1), chain b
            # partitions 64:128 (groups 2-3); the PE streams both
            # concurrently, halving the m=1 projection time. part=0/1
            # optionally spreads the chunks over two filler pumps.
            key = tuple(s[4] for s in specs)
            if part == 1:
                ps = kqp_state.pop(key)
            elif head:
                ps = psp.tile([128, 1024], F32, name="ps", tag="ps",
                              bufs=2)[:, 0:512]
            else:
                ps = pjp.tile([128, 512], F32, name="pj", tag="pj")
            if part == 0:
                kqp_state[key] = ps
            chunks = (range(0, 3) if part == 0
                      else range(3, NDC) if part == 1 else range(NDC))
            for kc in chunks:
                for half, (dst2, w_t, rhs, bc, n) in zip((0, 64), specs):
                    nc.tensor.matmul(
                        ps[half:half + 64, :],
                        lhsT=w_t[:, kc * DL:(kc + 1) * DL][:, 128:DL],
                        rhs=rhs[:, kc * 512:(kc + 1) * 512],
                        start=(kc == 0), stop=(kc == NDC - 1),
                        skip_group_check=True,
                    )
            if part == 0:
                return
            identf = mybir.ActivationFunctionType.Identity
            for half, (dst2, w_t, rhs, bc, n) in zip((0, 64), specs):
                nsl = slice(n * 512, (n + 1) * 512)
                for dhalf in (0, 64):
                    src = ps[half:half + 64, :]
                    dst = dst2[dhalf:dhalf + 64, nsl]
                    bias = bkq1[dhalf:dhalf + 64, bc:bc + 1]
                    if head:
                        nc.scalar.activation(out=dst, in_=src, func=identf,
                                             bias=bias)
                    else:
                        nc.vector.tensor_scalar_add(out=dst, in0=src,
                                                    scalar1=bias)

        def v_proj(kb, head=False):
            if head:
                pv = psp.tile([128, 1024], F32, name="ps", tag="ps",
                              bufs=2)[:, 0:512]
            else:
                pv = pjp.tile([128, 512], F32, name="pj", tag="pj")
            n, j = kb // 4, (kb % 4) * 128
            for kc in range(NDC):
                nc.tensor.matmul(
                    pv[:, :DL], lhsT=xt[n][:, kc * 512 + j:kc * 512 + j + 128],
                    rhs=wv_t[:, kc * DL:(kc + 1) * DL],
                    start=(kc == 0), stop=(kc == NDC - 1),
                )
            # one strided add writes all 3 heads' V cols (ones col skipped)
            voff = kb * HL * VW
            dstv = vbig[:, voff:voff + HL * VW]
            dstv = dstv.rearrange("p (h vw) -> p h vw", vw=VW)[:, :, 0:HD]
            nc.vector.tensor_add(
                out=dstv,
                in0=pv[:, :DL].rearrange("p (h d) -> p h d", d=HD),
                in1=bvb.rearrange("p (h d) -> p h d", d=HD),
            )

        # per-head score/attn tiles: (lhsT source, rhs source, aT dest)
        kq_src = (
            (kT01, qT01, (0, 64)),     # head 0: always low half
            (kT01, qT01, (64, 128)),   # head 1: always high half
            (kT2, qT2, None),          # head 2: half chosen per matmul
        )
        aT_of = (aT01[0:64], aT01[64:128], aT2[0:64])

        ot_tiles = {}

        def out_proj_half(qt, ncol, pot=None, reg=0, act_copy=False):
            # split at the PSUM-slot reuse boundary so the WAR wait on the
            # previous half's copy never stalls the PE mid-filler
            osl = slice(qt * 128, (qt + 1) * 128)
            if ncol == 0:
                ot_tiles[qt] = osb.tile([128, D], BF16, name="ot", tag="ot")
            ot = ot_tiles[qt]
            cw = 512 if ncol == 0 else 256
            csl = slice(ncol * 512, ncol * 512 + cw)
            final = pot is not None
            if not final:
                pot = pjp.tile([128, 512], F32, name="pj", tag="pj")
            psl = slice(reg * 512, reg * 512 + cw)
            nc.tensor.matmul(
                pot[:, psl], lhsT=aT01[:, osl], rhs=wo0[:, csl],
                start=True, stop=False, skip_group_check=True)
            nc.tensor.matmul(
                pot[:, psl], lhsT=aT2[0:64, osl], rhs=wo1[0:64, csl],
                start=False, stop=True, skip_group_check=True)
            if act_copy:  # tail: split evacuations across DVE and ACT
                nc.scalar.activation(out=ot[:, csl], in_=pot[:, psl],
                                     func=mybir.ActivationFunctionType.Copy)
            else:
                nc.vector.tensor_copy(out=ot[:, csl], in_=pot[:, psl])
            if final:
                nc.gpsimd.dma_start(out=out[osl, csl], in_=ot[:, csl])
            elif ncol == 1:
                nc.gpsimd.dma_start(out=out[osl, :], in_=ot)

        def attention(t, fillers, late_fillers=(), late_start=0):
            late_fillers = list(late_fillers)
            nseen = [0]

            def pump(k=1):
                for _ in range(k):
                    nseen[0] += 1
                    if fillers:
                        fillers.pop(0)()
                    elif late_fillers and nseen[0] > late_start:
                        late_fillers.pop(0)()
                    elif k == 1 and nseen[0] % 2 == 0:
                        # spare pump: tiny keep-warm matmul so partial PE
                        # idle never accumulates into a HAM MID window
                        # (re-throttle to 1.2 GHz costs ~2us per event)
                        nc.tensor.matmul(pwarm[0:64, 0:64], lhsT=warm,
                                         rhs=warm, start=True, stop=True,
                                         skip_group_check=True)

            qoff = t * QTW
            last_kb = 8 * t + BAND_PACKS[-1][-1]

            # entry = (half, head, kb, psum_off, width, q_start, band)
            # pack = (entries, exp_ranges, heads_finishing)
            packs = []
            # phase A: heads 0+1 paired on alternating PE row tiles
            for kb in range(0, 8 * t):
                packs.append((
                    [(0, 0, kb, 0, 512, 0, False),
                     (64, 1, kb, 512, 512, 0, False)],
                    ((0, 1024),), ()))
            for pr in BAND_PACKS[:-1]:
                ent = []
                offs = [0, 512]
                for b in pr:
                    w = 512 - 64 * b
                    for h in (0, 1):
                        ent.append((64 * h, h, 8 * t + b, offs[h], w,
                                    64 * b, True))
                        offs[h] += w
                packs.append((ent, ((0, 1024),), ()))
            packs.append((
                [(0, 0, 8 * t + 4, 0, 256, 256, True),
                 (64, 1, 8 * t + 4, 512, 256, 256, True)],
                ((0, 256), (512, 768)), (0, 1)))
            # phase B: head 2 alternating its two duplicated halves
            for kb in range(0, 8 * t, 2):
                packs.append((
                    [(0, 2, kb, 0, 512, 0, False),
                     (64, 2, kb + 1, 512, 512, 0, False)],
                    ((0, 1024),), ()))
            for i, prs in enumerate((((0,), (1, 7)), ((2, 6), (3, 5)))):
                ent = []
                for j, pr in enumerate(prs):
                    off = 512 * j
                    for b in pr:
                        w = 512 - 64 * b
                        ent.append((64 * ((2 * i + j) % 2), 2, 8 * t + b,
                                    off, w, 64 * b, True))
                        off += w
                packs.append((ent, ((0, 1024),), ()))
            packs.append((
                [(0, 2, 8 * t + 4, 0, 256, 256, True)],
                ((0, 256),), (2,)))

            po_of = {}

            def emit_pv(pack, eT):
                entries, _, fin = pack
                for (_half, h, kb, off, w, qs, _band) in entries:
                    if h not in po_of:
                        # lazy: the slot's previous reader (divide of the
                        # evicted head) must already be emitted for the WAR
                        po_of[h] = pop.tile([VW, 512], F32, name="po",
                                            tag="po")
                    voff = kb * HL * VW + h * VW
                    nc.tensor.matmul(
                        po_of[h][0:VW, qs:qs + w],
                        lhsT=vbig[:, voff:voff + VW],
                        rhs=eT[:, off:off + w],
                        start=(kb == 0), stop=(kb == last_kb),
                        skip_group_check=True,
                    )
                for h in fin:
                    divide(h)

            def divide(h):
                # divide by the softmax sum (row HD of po). The 1->64
                # partition broadcast of the reciprocal goes over a gpsimd
                # DMA (off the PE); the final tile-3 head-2 divide keeps
                # the PE broadcast matmul since it sits on the critical
                # tail and the PE is idle there anyway.
                po = po_of[h]
                if DIVIDE_BCAST and not (t == 3 and h == 2):
                    rec1 = rp.tile([1, 512], F32, name="rec1", tag="rec1")
                    nc.vector.reciprocal_approx_fast(out=rec1,
                                                     in_=po[HD:VW, :])
                    recb = rp.tile([64, 512], F32, name="recb", tag="recb")
                    nc.gpsimd.dma_start(out=recb,
                                        in_=rec1.partition_broadcast(64))
                else:
                    sums = rp.tile([1, 512], BF16, name="sums", tag="sums")
                    nc.vector.tensor_copy(out=sums, in_=po[HD:VW, :])
                    pb = pjp.tile([128, 512], F32, name="pj", tag="pj")
                    nc.tensor.matmul(pb[0:64, :], lhsT=ones1, rhs=sums,
                                     start=True, stop=True)
                    recb = rp.tile([64, 512], F32, name="recb", tag="recb")
                    nc.vector.reciprocal_approx_fast(out=recb,
                                                     in_=pb[0:64, :])
                nc.vector.tensor_mul(
                    out=aT_of[h][:, qoff:qoff + QTW], in0=po[0:HD, :],
                    in1=recb)
                if h == 2:  # mirror into the upper half for dual-stream
                    nc.vector.tensor_copy(
                        out=aT2[64:128, qoff:qoff + QTW],
                        in_=aT2[0:64, qoff:qoff + QTW])

            pend = None  # (pack, eT) whose PV is not yet emitted
            for pack in packs:
                entries, exp_ranges, _fin = pack
                ps = psp.tile([128, 1024], F32, name="ps", tag="ps", bufs=2)
                for (half, h, kb, off, w, qs, _band) in entries:
                    kT_h, qT_h, fixed = kq_src[h]
                    if fixed is not None:
                        hsl = slice(fixed[0], fixed[1])
                    else:
                        hsl = slice(half, half + 64)
                    nc.tensor.matmul(
                        ps[:, off:off + w],
                        lhsT=kT_h[hsl, kb * 128:(kb + 1) * 128],
                        rhs=qT_h[hsl, qoff + qs:qoff + QTW],
                        start=True, stop=True,
                    )
                eT = ep.tile([128, 1024], BF16, name="eT", tag="eT", bufs=3)
                for (r0, r1) in exp_ranges:
                    nc.scalar.activation(
                        out=eT[:, r0:r1], in_=ps[:, r0:r1],
                        func=mybir.ActivationFunctionType.Exp, scale=SCALE)
                for (_half, _h, kb, off, w, qs, band) in entries:
                    if band:  # zero the 64 partial cols of the triangle
                        nc.vector.tensor_mul(
                            out=eT[:, off:off + 64],
                            in0=eT[:, off:off + 64], in1=msk)
                pump(1)
                if pend is not None:
                    emit_pv(*pend)
                pend = (pack, eT)
            emit_pv(*pend)
            pump(len(fillers) + len(late_fillers))

        # wo0-part of the final 4 row-blocks' out-projection: runs as late
        # fillers inside tile 3's phase B (aT01 for t=3 is ready once heads
        # 0/1 finish at the end of phase A); wo1 + combine run in the
        # epilogue once aT2 lands.
        ot0 = {qt: o0p.tile([128, D], BF16, name=f"ot0_{qt}")
               for qt in (12, 13, 14, 15)}

        def out_proj_pair(qta, qtb, ncol):
            # two row-blocks' out-projection halves: the wo0 parts run
            # back-to-back, then the two K=64 wo1 parts dual-stream via
            # aT2's duplicated partition halves.
            cw = 512 if ncol == 0 else 256
            csl = slice(ncol * 512, ncol * 512 + cw)
            pots = []
            for qt in (qta, qtb):
                osl = slice(qt * 128, (qt + 1) * 128)
                if ncol == 0:
                    ot_tiles[qt] = osb.tile([128, D], BF16, name="ot",
                                            tag="ot")
                pot = pjp.tile([128, 512], F32, name="pj", tag="pj")
                nc.tensor.matmul(
                    pot[:, :cw], lhsT=aT01[:, osl], rhs=wo0[:, csl],
                    start=True, stop=False, skip_group_check=True)
                pots.append(pot)
            for i, qt in enumerate((qta, qtb)):
                osl = slice(qt * 128, (qt + 1) * 128)
                hsl = slice(64 * i, 64 * i + 64)
                nc.tensor.matmul(
                    pots[i][:, :cw], lhsT=aT2[hsl, osl], rhs=wo1[hsl, csl],
                    start=False, stop=True, skip_group_check=True)
            for i, qt in enumerate((qta, qtb)):
                osl = slice(qt * 128, (qt + 1) * 128)
                ot = ot_tiles[qt]
                nc.vector.tensor_copy(out=ot[:, csl], in_=pots[i][:, :cw])
                if ncol == 1:
                    nc.gpsimd.dma_start(out=out[osl, :], in_=ot)

        kqh_state = {}

        def kq_half(key, dst01, dst2, w_t, rhs, bc, n, m, part):
            # half of a 6-chunk projection: spreads one filler across two
            # pump slots so filler-heavy packs don't outrun the exp pace
            if part == 0:
                ps = pjp.tile([128, 512], F32, name="pj", tag="pj")
                kqh_state[key] = ps
            else:
                ps = kqh_state.pop(key)
            mw = 128 if m == 0 else 64
            msl = slice(0, 128) if m == 0 else slice(128, DL)
            for kc in (range(0, 3) if part == 0 else range(3, NDC)):
                nc.tensor.matmul(
                    ps[:mw, :],
                    lhsT=w_t[:, kc * DL:(kc + 1) * DL][:, msl],
                    rhs=rhs[:, kc * 512:(kc + 1) * 512],
                    start=(kc == 0), stop=(kc == NDC - 1),
                )
            if part == 1:
                nsl = slice(n * 512, (n + 1) * 512)
                nc.vector.tensor_scalar_add(out=dst01[:, nsl], in0=ps,
                                            scalar1=bkq0[:, bc:bc + 1])

        def wo0_half(qt, ncol):
            osl = slice(qt * 128, (qt + 1) * 128)
            cw = 512 if ncol == 0 else 256
            csl = slice(ncol * 512, ncol * 512 + cw)
            pw = pjp.tile([128, 512], F32, name="pj", tag="pj")
            nc.tensor.matmul(pw[:, :cw], lhsT=aT01[:, osl], rhs=wo0[:, csl],
                             start=True, stop=True, skip_group_check=True)
            nc.vector.tensor_copy(out=ot0[qt][:, csl], in_=pw[:, :cw])

        # ---- schedule: minimal head, then q-tiles t=0..3 with fillers ----
        def K(n, m):
            return lambda: kq_proj(kT01, kT2, wk_t, xt[n], 0, n, m)

        def Q(t, m):
            return lambda: kq_proj(qT01, qT2, wq_t, xq[t], 1, t, m)

        def V(kb):
            return lambda: v_proj(kb)

        def O(qt, ncol):
            return lambda: out_proj_half(qt, ncol)

        def W0(qt, ncol):
            return lambda: wo0_half(qt, ncol)

        def O2(qta, qtb, ncol):
            return lambda: out_proj_pair(qta, qtb, ncol)

        def KH(n, part):
            return lambda: kq_half(("k", n), kT01, kT2, wk_t, xt[n], 0,
                                   n, 0, part)

        def QH(t, part):
            return lambda: kq_half(("q", t), qT01, qT2, wq_t, xq[t], 1,
                                   t, 0, part)

        def KP1(na, nb, part=None):
            return lambda: kq1_pair([(kT2, wk_t, xt[na], 0, na),
                                     (kT2, wk_t, xt[nb], 0, nb)], part=part)

        def QP1(ta, tb):
            return lambda: kq1_pair([(qT2, wq_t, xq[ta], 1, ta),
                                     (qT2, wq_t, xq[tb], 1, tb)])

        # head: only what attention(0)'s phase A needs, DMA-ordered.
        # m=1 halves run as col-tiled pairs (same wall time as one); the
        # q-side m=1 pair rides tile-0's fillers (deadline: its phase B),
        # so the head never waits on the xq1 transfer.
        def hwarm(k=4):
            # bridge the head's DMA-wait pockets with dummy matmuls so the
            # HAM activity window keeps counting and the attention stream
            # enters already at 2.4 GHz
            for _ in range(k):
                nc.tensor.matmul(pwarm[0:64, 0:64], lhsT=warm, rhs=warm,
                                 start=True, stop=True,
                                 skip_group_check=True)

        kq_proj(kT01, kT2, wk_t, xt[0], 0, 0, 0, head=True)
        hwarm()
        v_proj(0, head=True)
        v_proj(1, head=True)
        hwarm(10)
        kq_proj(qT01, qT2, wq_t, xq[0], 1, 0, 0, head=True)
        hwarm(10)
        kq_proj(kT01, kT2, wk_t, xt[1], 0, 1, 0, head=True)
        kq1_pair([(kT2, wk_t, xt[0], 0, 0), (kT2, wk_t, xt[1], 0, 1)],
                 head=True)
        v_proj(7, head=True)

        # V fillers ordered by the band-block usage order of the next tile
        def Vband(t):
            return [V(8 * t + b) for b in (0, 1, 7, 2, 6, 3, 5, 4)]

        # Fillers sized to each tile's pack count (12t+8 pumps) so no tile
        # ends with a serialized dump; each tile's prerequisites (K/Q/V of
        # the NEXT tile) sit early enough in the list to land just in time.
        FILL = {
            0: [V(2), V(6), QP1(0, 1), V(3), V(5), V(4), Q(1, 0), K(2, 0)],
            1: [KH(3, 0), KH(3, 1), KP1(2, 3)] + Vband(1) +
               [KH(4, 0), KH(4, 1), KH(5, 0), KH(5, 1), KP1(4, 5),
                QH(2, 0), QH(2, 1), QP1(2, 3)],
            2: Vband(2) + [KH(6, 0), KH(6, 1), KH(7, 0), KH(7, 1),
                           KP1(6, 7, 0), KP1(6, 7, 1), QH(3, 0), QH(3, 1)],
            3: Vband(3) + [O2(0, 1, 0), O2(0, 1, 1), O2(2, 3, 0),
                           O2(2, 3, 1), O2(4, 5, 0), O2(4, 5, 1),
                           O2(6, 7, 0), O2(6, 7, 1), O2(8, 9, 0),
                           O2(8, 9, 1), O2(10, 11, 0), O2(10, 11, 1)],
        }
        LATE3 = [W0(12, 0), W0(12, 1), W0(13, 0), W0(13, 1),
                 W0(14, 0), W0(14, 1), W0(15, 0), W0(15, 1)]
        for t in range(NQT):
            if t == 3:
                attention(t, FILL[t], late_fillers=LATE3, late_start=8 * t + 6)
            else:
                attention(t, FILL[t])
        # epilogue: wo1-part of the final 4 row-blocks (needs aT2 of tile
        # 3). The staged wo0-part is re-injected into PSUM via an identity
        # matmul (PE is idle here), the wo1 pairs dual-stream via aT2's
        # duplicated halves, and a cheap 2x CAST copy replaces a DVE add.
        # Keep-warm dummies bridge the divide(2) stall so the epilogue
        # matmuls run at 2.4 GHz.
        for _ in range(12):
            nc.tensor.matmul(pwarm[0:64, 0:64], lhsT=warm, rhs=warm,
                             start=True, stop=True, skip_group_check=True)
        otf_of = {}
        for (qta, ncola), (qtb, ncolb) in (((12, 0), (13, 0)),
                                           ((14, 0), (15, 0)),
                                           ((12, 1), (13, 1)),
                                           ((14, 1), (15, 1))):
            pt = psp.tile([128, 1024], F32, name="ps", tag="ps", bufs=2)
            for i, (qt, ncol) in enumerate(((qta, ncola), (qtb, ncolb))):
                osl = slice(qt * 128, (qt + 1) * 128)
                cw = 512 if ncol == 0 else 256
                csl = slice(ncol * 512, ncol * 512 + cw)
                nc.tensor.matmul(
                    pt[:, 512 * i:512 * i + cw], lhsT=identp,
                    rhs=ot0[qt][:, csl], start=True, stop=False,
                    skip_group_check=True)
            for i, (qt, ncol) in enumerate(((qta, ncola), (qtb, ncolb))):
                osl = slice(qt * 128, (qt + 1) * 128)
                hsl = slice(64 * i, 64 * i + 64)
                cw = 512 if ncol == 0 else 256
                csl = slice(ncol * 512, ncol * 512 + cw)
                nc.tensor.matmul(
                    pt[:, 512 * i:512 * i + cw], lhsT=aT2[hsl, osl],
                    rhs=wo1[hsl, csl], start=False, stop=True,
                    skip_group_check=True)
            for i, (qt, ncol) in enumerate(((qta, ncola), (qtb, ncolb))):
                osl = slice(qt * 128, (qt + 1) * 128)
                cw = 512 if ncol == 0 else 256
                csl = slice(ncol * 512, ncol * 512 + cw)
                if qt not in otf_of:
                    otf_of[qt] = osb.tile([128, D], BF16, name="ot", tag="ot")
                otf = otf_of[qt]
                # evacuations and output DMAs alternate engines/queues so
                # the two per-pair chains drain concurrently
                if i == 0:
                    nc.scalar.activation(
                        out=otf[:, csl], in_=pt[:, 0:cw],
                        func=mybir.ActivationFunctionType.Copy)
                    nc.sync.dma_start(out=out[osl, csl], in_=otf[:, csl])
                else:
                    nc.vector.tensor_copy(out=otf[:, csl],
                                          in_=pt[:, 512:512 + cw])
                    nc.gpsimd.dma_start(out=out[osl, csl], in_=otf[:, csl])

    nc.finalize()
    return nc


_NC_CACHE = {}


def _get_nc():
    if "nc" not in _NC_CACHE:
        _NC_CACHE["nc"] = build_nc()
    return _NC_CACHE["nc"]


def kernel(x, Wqkv, bqkv, Wout, bout):
    x = np.asarray(x, dtype=np.float32)
    Wqkv = np.asarray(Wqkv, dtype=np.float32)
    bqkv = np.asarray(bqkv, dtype=np.float32)
    Wout = np.asarray(Wout, dtype=np.float32)
    bout = np.asarray(bout, dtype=np.float32)
    B, S_, D_ = x.shape
    assert (B, S_, D_) == (1, S, D)
    nc = _get_nc()

    xT_np = np.ascontiguousarray(x[0].T).astype(NPBF16)          # [768, 4096]

    def swizzle_x(a):
        # [768, ncols] -> [128, ntiles*6*512]: SBUF tile layout (tile-major,
        # then contraction chunk, then 512 cols) so device DMAs are dense.
        ncols = a.shape[1]
        nt = ncols // 512
        return np.ascontiguousarray(
            a.reshape(6, 128, nt, 512).transpose(1, 2, 0, 3).reshape(
                128, nt * 6 * 512))

    def swizzle_w(w):
        # [768, DL] -> [128, 6*DL]
        return np.ascontiguousarray(
            w.reshape(6, 128, DL).transpose(1, 0, 2).reshape(128, 6 * DL))

    xT_sw = swizzle_x(xT_np)
    xq_sw = [swizzle_x(np.ascontiguousarray(xT_np[:, p::2])) for p in (0, 1)]
    in_maps = []
    for c in range(8):
        g, p = c // 2, c % 2
        csl = slice(DL * g, DL * (g + 1))
        rr = np.arange(128, dtype=np.int64)[:, None]
        cc = np.arange(64, dtype=np.int64)[None, :]
        mask = (rr <= 2 * cc + p).astype(NPBF16)
        bk_h = bqkv[D + DL * g:D + DL * (g + 1)].astype(np.float32)
        bq_h = bqkv[csl].astype(np.float32)
        in_maps.append({
            "xT": xT_sw,
            "xqT": xq_sw[p],
            "wk": swizzle_w(Wqkv[:, D + DL * g:D + DL * (g + 1)].astype(NPBF16)),
            "wq": swizzle_w(Wqkv[:, csl].astype(NPBF16)),
            "wv": swizzle_w(Wqkv[:, 2 * D + DL * g:2 * D + DL * (g + 1)].astype(NPBF16)),
            "bkq": np.ascontiguousarray(np.stack([bk_h, bq_h], axis=1)),
            "bv": np.ascontiguousarray(bqkv[2 * D + DL * g:2 * D + DL * (g + 1)]).astype(np.float32),
            "wout": np.ascontiguousarray(Wout[csl, :]).astype(NPBF16),
            "mask64": mask,
            "ident": np.eye(128, dtype=NPBF16),
        })

    trace = bool(int(os.environ.get("ATTN_TRACE", "0")))
    tmpdir = os.environ.get("ATTN_TMPDIR") or None
    res = run_bass_kernel_spmd(nc, in_maps, core_ids=list(range(8)), trace=trace,
                               tmpdir=tmpdir)
    if trace:
        _NC_CACHE["last_result"] = res

    out_full = np.zeros((S, D), np.float32)
    for p in range(2):
        acc = np.zeros((SQ, D), np.float32)
        for g in range(4):
            acc += res.results[2 * g + p]["out"].astype(np.float32)
        out_full[p::2] = acc
    out_full += bout.astype(np.float32)[None, :]
    return out_full[None].astype(np.float32)



# revision 60
# speedup vs baseline: 1.1904x; 1.0004x over previous
"""Causal self-attention (B=1, S=4096, D=768, H=12) on 8 TRN2 NeuronCores.

Sharding: 4 head-groups (3 heads each) x 2 query-parity halves; no
collectives. Core c = 2*g + p handles heads [3g, 3g+3) and query rows
{r : r % 2 == p} (strided assignment balances causal work perfectly).

V4 highlights (on top of V3's dual-streamed scores, causal band
truncation, pack-interleaved exp, and filler-threaded projections):
  - All inputs arrive host-pre-swizzled into exact SBUF tile layouts so
    every input DMA is a dense [128, N] full-rate copy, issued on ONE
    queue in strict consumption order (HBM bw is shared across queues).
  - m=1 (64-row) K/Q projection halves run as col-tiled PAIRS (two M=64
    matmuls in one [128,512] psum, array col groups 0-1 / 2-3), halving
    their PE time; same trick dual-streams pairs of K=64 wo1 matmuls
    via aT2/wo1 duplicated into both partition halves.
  - Fillers are budgeted per tile (12t+8 pumps) with just-in-time
    deadlines so no projection ever dumps into a serialized ping-pong;
    6-chunk projections split across two pumps where packs are scarce.
  - PE preheat + keep-warm dummy matmuls hold the HAM clock gate at
    8/8 (2.4 GHz) through the ramp, spare pumps, and the epilogue.
  - The last 4 row-blocks' out-projection is split: the wo0 part runs
    as late fillers inside tile 3's phase B, the wo1 part + identity
    re-injection of the staged wo0 + CAST + DMA form a short epilogue
    with evacuations/DMAs alternated across engines/queues.

All matmuls run in bf16 (f32 PSUM accumulation); softmax exp in f32.
"""
import os

import numpy as np
import ml_dtypes

import concourse.bass as bass
import concourse.mybir as mybir
import concourse.tile as tile
from concourse import bacc
from concourse.bass_utils import run_bass_kernel_spmd

BF16 = mybir.dt.bfloat16
F32 = mybir.dt.float32
NPBF16 = ml_dtypes.bfloat16

S = 4096          # sequence length
D = 768           # model dim
HD = 64           # head dim
HL = 3            # heads per core
DL = HL * HD      # 192 local qkv cols per core
SQ = S // 2       # 2048 local queries per core
NQT = 4           # q-tiles per core
QTW = 512         # q-tile width (local queries)
NKB = S // 128    # 32 key blocks of 128
NDC = D // 128    # 6 contraction chunks of 128 over D
VW = HD + 1       # V' column stride per head (64 V cols + ones col)
SCALE = HD ** -0.5

# band packs: diagonal blocks b paired so each pack's widths sum to <=512
BAND_PACKS = ((0,), (1, 7), (2, 6), (3, 5), (4,))

# divide-by-softmax-sum via a partition-broadcast DVE read instead of a
# PE broadcast matmul (saves ~12 matmuls + a DVE op per divide)
DIVIDE_BCAST = False  # zero-step partition APs rejected on DVE and DMA paths


def build_nc():
    # xT / xqT / wk / wq / wv arrive pre-swizzled from the host into the
    # exact SBUF tile layouts, so every input DMA is a dense [128, N] copy
    # (full-rate contiguous segments, no strided descriptor storms).
    nc = bacc.Bacc(None, target_bir_lowering=False)
    xT = nc.declare_dram_parameter("xT", [128, 8 * NDC * 512], BF16,
                                   isOutput=False)
    xqT = nc.declare_dram_parameter("xqT", [128, 4 * NDC * 512], BF16,
                                    isOutput=False)
    wk = nc.declare_dram_parameter("wk", [128, NDC * DL], BF16, isOutput=False)
    wq = nc.declare_dram_parameter("wq", [128, NDC * DL], BF16, isOutput=False)
    wv = nc.declare_dram_parameter("wv", [128, NDC * DL], BF16, isOutput=False)
    bkq = nc.declare_dram_parameter("bkq", [DL, 2], F32, isOutput=False)
    bv = nc.declare_dram_parameter("bv", [DL], F32, isOutput=False)
    wout = nc.declare_dram_parameter("wout", [DL, D], BF16, isOutput=False)
    mask64 = nc.declare_dram_parameter("mask64", [128, 64], BF16, isOutput=False)
    ident = nc.declare_dram_parameter("ident", [128, 128], BF16, isOutput=False)
    out = nc.declare_dram_parameter("out", [SQ, D], BF16, isOutput=True)

    from contextlib import ExitStack

    with tile.TileContext(nc) as tc, ExitStack() as ctx:
        persist = ctx.enter_context(tc.tile_pool(name="persist", bufs=1))
        xtp = ctx.enter_context(tc.tile_pool(name="xtp", bufs=1))
        wp = ctx.enter_context(tc.tile_pool(name="wp", bufs=1))
        o0p = ctx.enter_context(tc.tile_pool(name="o0p", bufs=1))
        pjp = ctx.enter_context(tc.tile_pool(name="pjp", bufs=2, space="PSUM"))
        psp = ctx.enter_context(tc.tile_pool(name="psp", bufs=1, space="PSUM"))
        pop = ctx.enter_context(tc.tile_pool(name="pop", bufs=2, space="PSUM"))
        ep = ctx.enter_context(tc.tile_pool(name="ep", bufs=2))
        rp = ctx.enter_context(tc.tile_pool(name="rp", bufs=2))
        osb = ctx.enter_context(tc.tile_pool(name="osb", bufs=3))

        kT01 = persist.tile([128, S], BF16)         # K^T heads 0,1
        kT2 = persist.tile([128, S], BF16)          # K^T head 2 (both halves)
        qT01 = persist.tile([128, SQ], BF16)        # Q^T heads 0,1
        qT2 = persist.tile([128, SQ], BF16)         # Q^T head 2 (both halves)
        aT01 = persist.tile([128, SQ], BF16)        # attn^T heads 0,1
        aT2 = persist.tile([128, SQ], BF16)         # head 2, dup halves
        vbig = persist.tile([128, NKB * HL * VW], BF16)  # V' blocks [k,195]
        bvb = persist.tile([128, DL], F32)          # bv broadcast over rows
        msk = persist.tile([128, 64], BF16)         # causal triangle r<=2c+p
        identp = persist.tile([128, 128], BF16)     # identity (epilogue)
        ones1 = persist.tile([1, 64], BF16)
        bkq0 = persist.tile([128, 2], F32)
        bkq1 = persist.tile([128, 2], F32)          # [0:64] and [64:128] same
        wo0 = persist.tile([128, D], BF16)
        wo1 = persist.tile([128, D], BF16)  # wout[128:DL] duplicated in
        # both partition halves so two wo1 matmuls can dual-stream the PE

        warm = persist.tile([128, 64], BF16)
        nc.gpsimd.memset(warm, 0.125)
        # only V's softmax-sum columns (col 64 of each head's VW stride)
        # need the 1.0 fill; v_proj overwrites the rest. A strided memset
        # keeps the gpsimd queue free for the bias/mask DMAs the head needs.
        nc.gpsimd.memset(
            vbig.rearrange("p (k h v) -> p k h v", h=HL, v=VW)[:, :, :, HD:VW],
            1.0)
        nc.gpsimd.memset(ones1, 1.0)

        # x^T / xq^T land as 512-column slices holding all 6 contraction
        # chunks: tile cols = kc*512 + j. Weights land as [128, 6*DL].
        xt = [xtp.tile([128, NDC * 512], BF16, name=f"xt{n}") for n in range(8)]
        xq = [xtp.tile([128, NDC * 512], BF16, name=f"xq{t}") for t in range(NQT)]
        wk_t = wp.tile([128, NDC * DL], BF16, name="wk")
        wq_t = wp.tile([128, NDC * DL], BF16, name="wq")
        wv_t = wp.tile([128, NDC * DL], BF16, name="wv")

        # input DMAs: dense [128, N] copies (host pre-swizzled), spread
        # across the 3 DMA-capable queues, need-ordered within each queue.
        # xt[0]/xq[0] split by contraction chunk so the head's first
        # matmuls start as soon as their chunk lands.
        TW = NDC * 512  # 3072 cols per x tile

        def dma_x(xts, n, eng, c0=0, c1=NDC):
            eng.dma_start(out=xts[n][:, c0 * 512:c1 * 512],
                          in_=(xT if xts is xt else xqT)[
                              :, n * TW + c0 * 512:n * TW + c1 * 512])

        # Strict need-priority on ONE queue: HBM bandwidth (~360GB/s/core)
        # is shared across queues, so concurrent queues starve the
        # early-critical transfers. Everything big goes on sync in exact
        # consumption order; gpsimd carries only the small tensors.
        nc.sync.dma_start(out=wk_t, in_=wk[:, :])
        dma_x(xt, 0, nc.sync, 0, 2)
        dma_x(xt, 0, nc.sync, 2, 4)
        dma_x(xt, 0, nc.sync, 4, 6)
        nc.sync.dma_start(out=wv_t, in_=wv[:, :])
        nc.sync.dma_start(out=wq_t, in_=wq[:, :])
        dma_x(xq, 0, nc.sync, 0, 3)
        dma_x(xq, 0, nc.sync, 3, 6)
        dma_x(xt, 1, nc.sync)
        dma_x(xq, 1, nc.sync)
        dma_x(xt, 2, nc.sync)
        dma_x(xt, 3, nc.sync)
        dma_x(xt, 4, nc.sync)
        dma_x(xq, 2, nc.sync)
        dma_x(xq, 3, nc.sync)
        dma_x(xt, 5, nc.sync)
        dma_x(xt, 6, nc.sync)
        dma_x(xt, 7, nc.sync)

        # small tensors on the gpsimd queue
        nc.gpsimd.dma_start(out=bkq0, in_=bkq[0:128, :])
        nc.gpsimd.dma_start(out=bkq1[0:64, :], in_=bkq[128:DL, :])
        nc.gpsimd.dma_start(out=bkq1[64:128, :], in_=bkq[128:DL, :])
        nc.gpsimd.dma_start(out=bvb, in_=bv[:].partition_broadcast(128))
        nc.gpsimd.dma_start(out=msk, in_=mask64[:, :])
        nc.gpsimd.dma_start(out=wo0, in_=wout[0:128, :])
        nc.gpsimd.dma_start(out=wo1[0:64, :], in_=wout[128:DL, :])
        nc.gpsimd.dma_start(out=wo1[64:128, :], in_=wout[128:DL, :])
        nc.gpsimd.dma_start(out=identp, in_=ident[:, :])

        # PE preheat: tiny matmuls during the initial DMA wait so the HAM
        # clock-gate's busy window starts counting before the real head
        # projections begin (flip to 8/8 needs ~3.4us of sustained PE
        # activity).
        pwarm = pjp.tile([128, 512], F32, name="pwarm", tag="pj")
        for _ in range(12):
            nc.tensor.matmul(pwarm[0:64, 0:64], lhsT=warm, rhs=warm,
                             start=True, stop=True, skip_group_check=True)

        def kq_proj(dst01, dst2, w_t, rhs, bc, n, m, head=False):
            # dst[m-rows, cols n*512..] = W^T x^T + b  for one m-pass.
            # head=True: rotate through the (idle) score PSUM slots and
            # evacuate on the (idle) scalar engine - no single-slot WAR
            # stall, no DVE backlog.
            nsl = slice(n * 512, (n + 1) * 512)
            mw = 128 if m == 0 else 64
            msl = slice(0, 128) if m == 0 else slice(128, DL)
            if head:
                ps = psp.tile([128, 1024], F32, name="ps", tag="ps",
                              bufs=2)[:, 0:512]
            else:
                ps = pjp.tile([128, 512], F32, name="pj", tag="pj")
            for kc in range(NDC):
                nc.tensor.matmul(
                    ps[:mw, :],
                    lhsT=w_t[:, kc * DL:(kc + 1) * DL][:, msl],
                    rhs=rhs[:, kc * 512:(kc + 1) * 512],
                    start=(kc == 0), stop=(kc == NDC - 1),
                )
            ident = mybir.ActivationFunctionType.Identity

            def evac(dst, src, bias):
                if head:
                    nc.scalar.activation(out=dst, in_=src, func=ident,
                                         bias=bias)
                else:
                    nc.vector.tensor_scalar_add(out=dst, in0=src,
                                                scalar1=bias)

            if m == 0:
                evac(dst01[:, nsl], ps, bkq0[:, bc:bc + 1])
            else:  # head 2: write both partition halves (dual-tile scores)
                evac(dst2[0:64, nsl], ps[:64, :], bkq1[0:64, bc:bc + 1])
                evac(dst2[64:128, nsl], ps[:64, :], bkq1[64:128, bc:bc + 1])

        kqp_state = {}

        def kq1_pair(specs, head=False, part=None):
            # Two m=1 (64-row) projections col-tiled side by side: chain a
            # writes psum partitions 0:64 (array col groups 0-1), chain b
            # partitions 64:128 (groups 2-3); the PE streams both
            # concurrently, halving the m=1 projection time. part=0/1
            # optionally spreads the chunks over two filler pumps.
            key = tuple(s[4] for s in specs)
            if part == 1:
                ps = kqp_state.pop(key)
            elif head:
                ps = psp.tile([128, 1024], F32, name="ps", tag="ps",
                              bufs=2)[:, 0:512]
            else:
                ps = pjp.tile([128, 512], F32, name="pj", tag="pj")
            if part == 0:
                kqp_state[key] = ps
            chunks = (range(0, 3) if part == 0
                      else range(3, NDC) if part == 1 else range(NDC))
            for kc in chunks:
                for half, (dst2, w_t, rhs, bc, n) in zip((0, 64), specs):
                    nc.tensor.matmul(
                        ps[half:half + 64, :],
                        lhsT=w_t[:, kc * DL:(kc + 1) * DL][:, 128:DL],
                        rhs=rhs[:, kc * 512:(kc + 1) * 512],
                        start=(kc == 0), stop=(kc == NDC - 1),
                        skip_group_check=True,
                    )
            if part == 0:
                return
            identf = mybir.ActivationFunctionType.Identity
            for half, (dst2, w_t, rhs, bc, n) in zip((0, 64), specs):
                nsl = slice(n * 512, (n + 1) * 512)
                for dhalf in (0, 64):
                    src = ps[half:half + 64, :]
                    dst = dst2[dhalf:dhalf + 64, nsl]
                    bias = bkq1[dhalf:dhalf + 64, bc:bc + 1]
                    if head:
                        nc.scalar.activation(out=dst, in_=src, func=identf,
                                             bias=bias)
                    else:
                        nc.vector.tensor_scalar_add(out=dst, in0=src,
                                                    scalar1=bias)

        def v_proj(kb, head=False):
            if head:
                pv = psp.tile([128, 1024], F32, name="ps", tag="ps",
                              bufs=2)[:, 0:512]
            else:
                pv = pjp.tile([128, 512], F32, name="pj", tag="pj")
            n, j = kb // 4, (kb % 4) * 128
            for kc in range(NDC):
                nc.tensor.matmul(
                    pv[:, :DL], lhsT=xt[n][:, kc * 512 + j:kc * 512 + j + 128],
                    rhs=wv_t[:, kc * DL:(kc + 1) * DL],
                    start=(kc == 0), stop=(kc == NDC - 1),
                )
            # one strided add writes all 3 heads' V cols (ones col skipped)
            voff = kb * HL * VW
            dstv = vbig[:, voff:voff + HL * VW]
            dstv = dstv.rearrange("p (h vw) -> p h vw", vw=VW)[:, :, 0:HD]
            nc.vector.tensor_add(
                out=dstv,
                in0=pv[:, :DL].rearrange("p (h d) -> p h d", d=HD),
                in1=bvb.rearrange("p (h d) -> p h d", d=HD),
            )

        # per-head score/attn tiles: (lhsT source, rhs source, aT dest)
        kq_src = (
            (kT01, qT01, (0, 64)),     # head 0: always low half
            (kT01, qT01, (64, 128)),   # head 1: always high half
            (kT2, qT2, None),          # head 2: half chosen per matmul
        )
        aT_of = (aT01[0:64], aT01[64:128], aT2[0:64])

        ot_tiles = {}

        def out_proj_half(qt, ncol, pot=None, reg=0, act_copy=False):
            # split at the PSUM-slot reuse boundary so the WAR wait on the
            # previous half's copy never stalls the PE mid-filler
            osl = slice(qt * 128, (qt + 1) * 128)
            if ncol == 0:
                ot_tiles[qt] = osb.tile([128, D], BF16, name="ot", tag="ot")
            ot = ot_tiles[qt]
            cw = 512 if ncol == 0 else 256
            csl = slice(ncol * 512, ncol * 512 + cw)
            final = pot is not None
            if not final:
                pot = pjp.tile([128, 512], F32, name="pj", tag="pj")
            psl = slice(reg * 512, reg * 512 + cw)
            nc.tensor.matmul(
                pot[:, psl], lhsT=aT01[:, osl], rhs=wo0[:, csl],
                start=True, stop=False, skip_group_check=True)
            nc.tensor.matmul(
                pot[:, psl], lhsT=aT2[0:64, osl], rhs=wo1[0:64, csl],
                start=False, stop=True, skip_group_check=True)
            if act_copy:  # tail: split evacuations across DVE and ACT
                nc.scalar.activation(out=ot[:, csl], in_=pot[:, psl],
                                     func=mybir.ActivationFunctionType.Copy)
            else:
                nc.vector.tensor_copy(out=ot[:, csl], in_=pot[:, psl])
            if final:
                nc.gpsimd.dma_start(out=out[osl, csl], in_=ot[:, csl])
            elif ncol == 1:
                nc.gpsimd.dma_start(out=out[osl, :], in_=ot)

        def attention(t, fillers, late_fillers=(), late_start=0,
                      tail_hook=None):
            late_fillers = list(late_fillers)
            nseen = [0]

            def pump(k=1):
                for _ in range(k):
                    nseen[0] += 1
                    if fillers:
                        fillers.pop(0)()
                    elif late_fillers and nseen[0] > late_start:
                        late_fillers.pop(0)()
                    elif k == 1 and nseen[0] % 2 == 0:
                        # spare pump: tiny keep-warm matmul so partial PE
                        # idle never accumulates into a HAM MID window
                        # (re-throttle to 1.2 GHz costs ~2us per event)
                        nc.tensor.matmul(pwarm[0:64, 0:64], lhsT=warm,
                                         rhs=warm, start=True, stop=True,
                                         skip_group_check=True)

            qoff = t * QTW
            last_kb = 8 * t + BAND_PACKS[-1][-1]

            # entry = (half, head, kb, psum_off, width, q_start, band)
            # pack = (entries, exp_ranges, heads_finishing)
            packs = []
            # phase A: heads 0+1 paired on alternating PE row tiles
            for kb in range(0, 8 * t):
                packs.append((
                    [(0, 0, kb, 0, 512, 0, False),
                     (64, 1, kb, 512, 512, 0, False)],
                    ((0, 1024),), ()))
            for pr in BAND_PACKS[:-1]:
                ent = []
                offs = [0, 512]
                for b in pr:
                    w = 512 - 64 * b
                    for h in (0, 1):
                        ent.append((64 * h, h, 8 * t + b, offs[h], w,
                                    64 * b, True))
                        offs[h] += w
                packs.append((ent, ((0, 1024),), ()))
            packs.append((
                [(0, 0, 8 * t + 4, 0, 256, 256, True),
                 (64, 1, 8 * t + 4, 512, 256, 256, True)],
                ((0, 256), (512, 768)), (0, 1)))
            # phase B: head 2 alternating its two duplicated halves
            for kb in range(0, 8 * t, 2):
                packs.append((
                    [(0, 2, kb, 0, 512, 0, False),
                     (64, 2, kb + 1, 512, 512, 0, False)],
                    ((0, 1024),), ()))
            for i, prs in enumerate((((0,), (1, 7)), ((2, 6), (3, 5)))):
                ent = []
                for j, pr in enumerate(prs):
                    off = 512 * j
                    for b in pr:
                        w = 512 - 64 * b
                        ent.append((64 * ((2 * i + j) % 2), 2, 8 * t + b,
                                    off, w, 64 * b, True))
                        off += w
                packs.append((ent, ((0, 1024),), ()))
            packs.append((
                [(0, 2, 8 * t + 4, 0, 256, 256, True)],
                ((0, 256),), (2,)))

            po_of = {}

            def emit_pv(pack, eT):
                entries, _, fin = pack
                for (_half, h, kb, off, w, qs, _band) in entries:
                    if h not in po_of:
                        # lazy: the slot's previous reader (divide of the
                        # evicted head) must already be emitted for the WAR
                        po_of[h] = pop.tile([VW, 512], F32, name="po",
                                            tag="po")
                    voff = kb * HL * VW + h * VW
                    nc.tensor.matmul(
                        po_of[h][0:VW, qs:qs + w],
                        lhsT=vbig[:, voff:voff + VW],
                        rhs=eT[:, off:off + w],
                        start=(kb == 0), stop=(kb == last_kb),
                        skip_group_check=True,
                    )
                if tail_hook is not None and 2 in fin:
                    # last pack of the last tile: emit PE work that does
                    # not depend on the final divide, so it overlaps the
                    # divide's DVE chain instead of queuing behind the
                    # broadcast matmul
                    tail_hook()
                for h in fin:
                    divide(h)

            def divide(h):
                # divide by the softmax sum (row HD of po). The 1->64
                # partition broadcast of the reciprocal goes over a gpsimd
                # DMA (off the PE); the final tile-3 head-2 divide keeps
                # the PE broadcast matmul since it sits on the critical
                # tail and the PE is idle there anyway.
                po = po_of[h]
                if DIVIDE_BCAST and not (t == 3 and h == 2):
                    rec1 = rp.tile([1, 512], F32, name="rec1", tag="rec1")
                    nc.vector.reciprocal_approx_fast(out=rec1,
                                                     in_=po[HD:VW, :])
                    recb = rp.tile([64, 512], F32, name="recb", tag="recb")
                    nc.gpsimd.dma_start(out=recb,
                                        in_=rec1.partition_broadcast(64))
                else:
                    sums = rp.tile([1, 512], BF16, name="sums", tag="sums")
                    nc.vector.tensor_copy(out=sums, in_=po[HD:VW, :])
                    pb = pjp.tile([128, 512], F32, name="pj", tag="pj")
                    nc.tensor.matmul(pb[0:64, :], lhsT=ones1, rhs=sums,
                                     start=True, stop=True)
                    recb = rp.tile([64, 512], F32, name="recb", tag="recb")
                    nc.vector.reciprocal_approx_fast(out=recb,
                                                     in_=pb[0:64, :])
                nc.vector.tensor_mul(
                    out=aT_of[h][:, qoff:qoff + QTW], in0=po[0:HD, :],
                    in1=recb)
                if h == 2:  # mirror into the upper half for dual-stream
                    nc.vector.tensor_copy(
                        out=aT2[64:128, qoff:qoff + QTW],
                        in_=aT2[0:64, qoff:qoff + QTW])

            pend = None  # (pack, eT) whose PV is not yet emitted
            for pack in packs:
                entries, exp_ranges, _fin = pack
                ps = psp.tile([128, 1024], F32, name="ps", tag="ps", bufs=2)
                for (half, h, kb, off, w, qs, _band) in entries:
                    kT_h, qT_h, fixed = kq_src[h]
                    if fixed is not None:
                        hsl = slice(fixed[0], fixed[1])
                    else:
                        hsl = slice(half, half + 64)
                    nc.tensor.matmul(
                        ps[:, off:off + w],
                        lhsT=kT_h[hsl, kb * 128:(kb + 1) * 128],
                        rhs=qT_h[hsl, qoff + qs:qoff + QTW],
                        start=True, stop=True,
                    )
                eT = ep.tile([128, 1024], BF16, name="eT", tag="eT", bufs=3)
                for (r0, r1) in exp_ranges:
                    nc.scalar.activation(
                        out=eT[:, r0:r1], in_=ps[:, r0:r1],
                        func=mybir.ActivationFunctionType.Exp, scale=SCALE)
                for (_half, _h, kb, off, w, qs, band) in entries:
                    if band:  # zero the 64 partial cols of the triangle
                        nc.vector.tensor_mul(
                            out=eT[:, off:off + 64],
                            in0=eT[:, off:off + 64], in1=msk)
                pump(1)
                if pend is not None:
                    emit_pv(*pend)
                pend = (pack, eT)
            emit_pv(*pend)
            pump(len(fillers) + len(late_fillers))

        # wo0-part of the final 4 row-blocks' out-projection: runs as late
        # fillers inside tile 3's phase B (aT01 for t=3 is ready once heads
        # 0/1 finish at the end of phase A); wo1 + combine run in the
        # epilogue once aT2 lands.
        ot0 = {qt: o0p.tile([128, D], BF16, name=f"ot0_{qt}")
               for qt in (12, 13, 14, 15)}

        def out_proj_pair(qta, qtb, ncol):
            # two row-blocks' out-projection halves: the wo0 parts run
            # back-to-back, then the two K=64 wo1 parts dual-stream via
            # aT2's duplicated partition halves.
            cw = 512 if ncol == 0 else 256
            csl = slice(ncol * 512, ncol * 512 + cw)
            pots = []
            for qt in (qta, qtb):
                osl = slice(qt * 128, (qt + 1) * 128)
                if ncol == 0:
                    ot_tiles[qt] = osb.tile([128, D], BF16, name="ot",
                                            tag="ot")
                pot = pjp.tile([128, 512], F32, name="pj", tag="pj")
                nc.tensor.matmul(
                    pot[:, :cw], lhsT=aT01[:, osl], rhs=wo0[:, csl],
                    start=True, stop=False, skip_group_check=True)
                pots.append(pot)
            for i, qt in enumerate((qta, qtb)):
                osl = slice(qt * 128, (qt + 1) * 128)
                hsl = slice(64 * i, 64 * i + 64)
                nc.tensor.matmul(
                    pots[i][:, :cw], lhsT=aT2[hsl, osl], rhs=wo1[hsl, csl],
                    start=False, stop=True, skip_group_check=True)
            for i, qt in enumerate((qta, qtb)):
                osl = slice(qt * 128, (qt + 1) * 128)
                ot = ot_tiles[qt]
                nc.vector.tensor_copy(out=ot[:, csl], in_=pots[i][:, :cw])
                if ncol == 1:
                    nc.gpsimd.dma_start(out=out[osl, :], in_=ot)

        kqh_state = {}

        def kq_half(key, dst01, dst2, w_t, rhs, bc, n, m, part):
            # half of a 6-chunk projection: spreads one filler across two
            # pump slots so filler-heavy packs don't outrun the exp pace
            if part == 0:
                ps = pjp.tile([128, 512], F32, name="pj", tag="pj")
                kqh_state[key] = ps
            else:
                ps = kqh_state.pop(key)
            mw = 128 if m == 0 else 64
            msl = slice(0, 128) if m == 0 else slice(128, DL)
            for kc in (range(0, 3) if part == 0 else range(3, NDC)):
                nc.tensor.matmul(
                    ps[:mw, :],
                    lhsT=w_t[:, kc * DL:(kc + 1) * DL][:, msl],
                    rhs=rhs[:, kc * 512:(kc + 1) * 512],
                    start=(kc == 0), stop=(kc == NDC - 1),
                )
            if part == 1:
                nsl = slice(n * 512, (n + 1) * 512)
                nc.vector.tensor_scalar_add(out=dst01[:, nsl], in0=ps,
                                            scalar1=bkq0[:, bc:bc + 1])

        def wo0_half(qt, ncol):
            osl = slice(qt * 128, (qt + 1) * 128)
            cw = 512 if ncol == 0 else 256
            csl = slice(ncol * 512, ncol * 512 + cw)
            pw = pjp.tile([128, 512], F32, name="pj", tag="pj")
            nc.tensor.matmul(pw[:, :cw], lhsT=aT01[:, osl], rhs=wo0[:, csl],
                             start=True, stop=True, skip_group_check=True)
            nc.vector.tensor_copy(out=ot0[qt][:, csl], in_=pw[:, :cw])

        # ---- schedule: minimal head, then q-tiles t=0..3 with fillers ----
        def K(n, m):
            return lambda: kq_proj(kT01, kT2, wk_t, xt[n], 0, n, m)

        def Q(t, m):
            return lambda: kq_proj(qT01, qT2, wq_t, xq[t], 1, t, m)

        def V(kb):
            return lambda: v_proj(kb)

        def O(qt, ncol):
            return lambda: out_proj_half(qt, ncol)

        def W0(qt, ncol):
            return lambda: wo0_half(qt, ncol)

        def O2(qta, qtb, ncol):
            return lambda: out_proj_pair(qta, qtb, ncol)

        def KH(n, part):
            return lambda: kq_half(("k", n), kT01, kT2, wk_t, xt[n], 0,
                                   n, 0, part)

        def QH(t, part):
            return lambda: kq_half(("q", t), qT01, qT2, wq_t, xq[t], 1,
                                   t, 0, part)

        def KP1(na, nb, part=None):
            return lambda: kq1_pair([(kT2, wk_t, xt[na], 0, na),
                                     (kT2, wk_t, xt[nb], 0, nb)], part=part)

        def QP1(ta, tb):
            return lambda: kq1_pair([(qT2, wq_t, xq[ta], 1, ta),
                                     (qT2, wq_t, xq[tb], 1, tb)])

        # head: only what attention(0)'s phase A needs, DMA-ordered.
        # m=1 halves run as col-tiled pairs (same wall time as one); the
        # q-side m=1 pair rides tile-0's fillers (deadline: its phase B),
        # so the head never waits on the xq1 transfer.
        def hwarm(k=4):
            # bridge the head's DMA-wait pockets with dummy matmuls so the
            # HAM activity window keeps counting and the attention stream
            # enters already at 2.4 GHz
            for _ in range(k):
                nc.tensor.matmul(pwarm[0:64, 0:64], lhsT=warm, rhs=warm,
                                 start=True, stop=True,
                                 skip_group_check=True)

        kq_proj(kT01, kT2, wk_t, xt[0], 0, 0, 0, head=True)
        hwarm()
        v_proj(0, head=True)
        v_proj(1, head=True)
        hwarm(10)
        kq_proj(qT01, qT2, wq_t, xq[0], 1, 0, 0, head=True)
        hwarm(10)
        kq_proj(kT01, kT2, wk_t, xt[1], 0, 1, 0, head=True)
        kq1_pair([(kT2, wk_t, xt[0], 0, 0), (kT2, wk_t, xt[1], 0, 1)],
                 head=True)
        v_proj(7, head=True)

        # V fillers ordered by the band-block usage order of the next tile
        def Vband(t):
            return [V(8 * t + b) for b in (0, 1, 7, 2, 6, 3, 5, 4)]

        # Fillers sized to each tile's pack count (12t+8 pumps) so no tile
        # ends with a serialized dump; each tile's prerequisites (K/Q/V of
        # the NEXT tile) sit early enough in the list to land just in time.
        FILL = {
            0: [V(2), V(6), QP1(0, 1), V(3), V(5), V(4), Q(1, 0), K(2, 0)],
            1: [KH(3, 0), KH(3, 1), KP1(2, 3)] + Vband(1) +
               [KH(4, 0), KH(4, 1), KH(5, 0), KH(5, 1), KP1(4, 5),
                QH(2, 0), QH(2, 1), QP1(2, 3)],
            2: Vband(2) + [KH(6, 0), KH(6, 1), KH(7, 0), KH(7, 1),
                           KP1(6, 7, 0), KP1(6, 7, 1), QH(3, 0), QH(3, 1)],
            3: Vband(3) + [O2(0, 1, 0), O2(0, 1, 1), O2(2, 3, 0),
                           O2(2, 3, 1), O2(4, 5, 0), O2(4, 5, 1),
                           O2(6, 7, 0), O2(6, 7, 1), O2(8, 9, 0),
                           O2(8, 9, 1), O2(10, 11, 0), O2(10, 11, 1)],
        }
        LATE3 = [W0(12, 0), W0(12, 1), W0(13, 0), W0(13, 1),
                 W0(14, 0), W0(14, 1), W0(15, 0), W0(15, 1)]
        EPAIRS = (((12, 0), (13, 0)), ((14, 0), (15, 0)),
                  ((12, 1), (13, 1)), ((14, 1), (15, 1)))
        ep_pts = []

        def ep_ident(pair):
            # wo0 re-injection for one epilogue pair; depends only on the
            # staged ot0 tiles and a free score-psum slot
            pt = psp.tile([128, 1024], F32, name="ps", tag="ps", bufs=2)
            for i, (qt, ncol) in enumerate(pair):
                cw = 512 if ncol == 0 else 256
                csl = slice(ncol * 512, ncol * 512 + cw)
                nc.tensor.matmul(
                    pt[:, 512 * i:512 * i + cw], lhsT=identp,
                    rhs=ot0[qt][:, csl], start=True, stop=False,
                    skip_group_check=True)
            ep_pts.append(pt)

        def tail3():
            ep_ident(EPAIRS[0])
            ep_ident(EPAIRS[1])

        for t in range(NQT):
            if t == 3:
                attention(t, FILL[t], late_fillers=LATE3,
                          late_start=8 * t + 6, tail_hook=tail3)
            else:
                attention(t, FILL[t])
        # epilogue: wo1-part of the final 4 row-blocks (needs aT2 of tile
        # 3). The staged wo0-part is re-injected into PSUM via an identity
        # matmul (PE is idle here), the wo1 pairs dual-stream via aT2's
        # duplicated halves, and a cheap 2x CAST copy replaces a DVE add.
        # Keep-warm dummies bridge the divide(2) stall so the epilogue
        # matmuls run at 2.4 GHz.
        for _ in range(4):
            nc.tensor.matmul(pwarm[0:64, 0:64], lhsT=warm, rhs=warm,
                             start=True, stop=True, skip_group_check=True)
        otf_of = {}
        for ip, ((qta, ncola), (qtb, ncolb)) in enumerate(EPAIRS):
            if ip < len(ep_pts):
                pt = ep_pts[ip]  # wo0 already injected inside tile 3
            else:
                ep_ident(EPAIRS[ip])
                pt = ep_pts[ip]
            for i, (qt, ncol) in enumerate(((qta, ncola), (qtb, ncolb))):
                osl = slice(qt * 128, (qt + 1) * 128)
                hsl = slice(64 * i, 64 * i + 64)
                cw = 512 if ncol == 0 else 256
                csl = slice(ncol * 512, ncol * 512 + cw)
                nc.tensor.matmul(
                    pt[:, 512 * i:512 * i + cw], lhsT=aT2[hsl, osl],
                    rhs=wo1[hsl, csl], start=False, stop=True,
                    skip_group_check=True)
            for i, (qt, ncol) in enumerate(((qta, ncola), (qtb, ncolb))):
                osl = slice(qt * 128, (qt + 1) * 128)
                cw = 512 if ncol == 0 else 256
                csl = slice(ncol * 512, ncol * 512 + cw)
                if qt not in otf_of:
                    otf_of[qt] = osb.tile([128, D], BF16, name="ot", tag="ot")
                otf = otf_of[qt]
                # evacuations and output DMAs alternate engines/queues so
                # the two per-pair chains drain concurrently
                if i == 0:
                    nc.scalar.activation(
                        out=otf[:, csl], in_=pt[:, 0:cw],
                        func=mybir.ActivationFunctionType.Copy)
                    nc.sync.dma_start(out=out[osl, csl], in_=otf[:, csl])
                else:
                    nc.vector.tensor_copy(out=otf[:, csl],
                                          in_=pt[:, 512:512 + cw])
                    nc.gpsimd.dma_start(out=out[osl, csl], in_=otf[:, csl])

    nc.finalize()
    return nc


_NC_CACHE = {}


def _get_nc():
    if "nc" not in _NC_CACHE:
        _NC_CACHE["nc"] = build_nc()
    return _NC_CACHE["nc"]


def kernel(x, Wqkv, bqkv, Wout, bout):
    x = np.asarray(x, dtype=np.float32)
    Wqkv = np.asarray(Wqkv, dtype=np.float32)
    bqkv = np.asarray(bqkv, dtype=np.float32)
    Wout = np.asarray(Wout, dtype=np.float32)
    bout = np.asarray(bout, dtype=np.float32)
    B, S_, D_ = x.shape
    assert (B, S_, D_) == (1, S, D)
    nc = _get_nc()

    xT_np = np.ascontiguousarray(x[0].T).astype(NPBF16)          # [768, 4096]

    def swizzle_x(a):
        # [768, ncols] -> [128, ntiles*6*512]: SBUF tile layout (tile-major,
        # then contraction chunk, then 512 cols) so device DMAs are dense.
        ncols = a.shape[1]
        nt = ncols // 512
        return np.ascontiguousarray(
            a.reshape(6, 128, nt, 512).transpose(1, 2, 0, 3).reshape(
                128, nt * 6 * 512))

    def swizzle_w(w):
        # [768, DL] -> [128, 6*DL]
        return np.ascontiguousarray(
            w.reshape(6, 128, DL).transpose(1, 0, 2).reshape(128, 6 * DL))

    xT_sw = swizzle_x(xT_np)
    xq_sw = [swizzle_x(np.ascontiguousarray(xT_np[:, p::2])) for p in (0, 1)]
    in_maps = []
    for c in range(8):
        g, p = c // 2, c % 2
        csl = slice(DL * g, DL * (g + 1))
        rr = np.arange(128, dtype=np.int64)[:, None]
        cc = np.arange(64, dtype=np.int64)[None, :]
        mask = (rr <= 2 * cc + p).astype(NPBF16)
        bk_h = bqkv[D + DL * g:D + DL * (g + 1)].astype(np.float32)
        bq_h = bqkv[csl].astype(np.float32)
        in_maps.append({
            "xT": xT_sw,
            "xqT": xq_sw[p],
            "wk": swizzle_w(Wqkv[:, D + DL * g:D + DL * (g + 1)].astype(NPBF16)),
            "wq": swizzle_w(Wqkv[:, csl].astype(NPBF16)),
            "wv": swizzle_w(Wqkv[:, 2 * D + DL * g:2 * D + DL * (g + 1)].astype(NPBF16)),
            "bkq": np.ascontiguousarray(np.stack([bk_h, bq_h], axis=1)),
            "bv": np.ascontiguousarray(bqkv[2 * D + DL * g:2 * D + DL * (g + 1)]).astype(np.float32),
            "wout": np.ascontiguousarray(Wout[csl, :]).astype(NPBF16),
            "mask64": mask,
            "ident": np.eye(128, dtype=NPBF16),
        })

    trace = bool(int(os.environ.get("ATTN_TRACE", "0")))
    tmpdir = os.environ.get("ATTN_TMPDIR") or None
    res = run_bass_kernel_spmd(nc, in_maps, core_ids=list(range(8)), trace=trace,
                               tmpdir=tmpdir)
    if trace:
        _NC_CACHE["last_result"] = res

    out_full = np.zeros((S, D), np.float32)
    for p in range(2):
        acc = np.zeros((SQ, D), np.float32)
        for g in range(4):
            acc += res.results[2 * g + p]["out"].astype(np.float32)
        out_full[p::2] = acc
    out_full += bout.astype(np.float32)[None, :]
    return out_full[None].astype(np.float32)



# revision 66
# speedup vs baseline: 1.1951x; 1.0039x over previous
"""Causal self-attention (B=1, S=4096, D=768, H=12) on 8 TRN2 NeuronCores.

Sharding: 4 head-groups (3 heads each) x 2 query-parity halves; no
collectives. Core c = 2*g + p handles heads [3g, 3g+3) and query rows
{r : r % 2 == p} (strided assignment balances causal work perfectly).

V4 highlights (on top of V3's dual-streamed scores, causal band
truncation, pack-interleaved exp, and filler-threaded projections):
  - All inputs arrive host-pre-swizzled into exact SBUF tile layouts so
    every input DMA is a dense [128, N] full-rate copy, issued on ONE
    queue in strict consumption order (HBM bw is shared across queues).
  - m=1 (64-row) K/Q projection halves run as col-tiled PAIRS (two M=64
    matmuls in one [128,512] psum, array col groups 0-1 / 2-3), halving
    their PE time; same trick dual-streams pairs of K=64 wo1 matmuls
    via aT2/wo1 duplicated into both partition halves.
  - Fillers are budgeted per tile (12t+8 pumps) with just-in-time
    deadlines so no projection ever dumps into a serialized ping-pong;
    6-chunk projections split across two pumps where packs are scarce.
  - PE preheat + keep-warm dummy matmuls hold the HAM clock gate at
    8/8 (2.4 GHz) through the ramp, spare pumps, and the epilogue.
  - The last 4 row-blocks' out-projection is split: the wo0 part runs
    as late fillers inside tile 3's phase B, the wo1 part + identity
    re-injection of the staged wo0 + CAST + DMA form a short epilogue
    with evacuations/DMAs alternated across engines/queues.

All matmuls run in bf16 (f32 PSUM accumulation); softmax exp in f32.
"""
import os

import numpy as np
import ml_dtypes

import concourse.bass as bass
import concourse.mybir as mybir
import concourse.tile as tile
from concourse import bacc
from concourse.bass_utils import run_bass_kernel_spmd

BF16 = mybir.dt.bfloat16
F32 = mybir.dt.float32
NPBF16 = ml_dtypes.bfloat16

S = 4096          # sequence length
D = 768           # model dim
HD = 64           # head dim
HL = 3            # heads per core
DL = HL * HD      # 192 local qkv cols per core
SQ = S // 2       # 2048 local queries per core
NQT = 4           # q-tiles per core
QTW = 512         # q-tile width (local queries)
NKB = S // 128    # 32 key blocks of 128
NDC = D // 128    # 6 contraction chunks of 128 over D
VW = HD + 1       # V' column stride per head (64 V cols + ones col)
SCALE = HD ** -0.5

# band packs: diagonal blocks b paired so each pack's widths sum to <=512
BAND_PACKS = ((0,), (1, 7), (2, 6), (3, 5), (4,))

# divide-by-softmax-sum via a partition-broadcast DVE read instead of a
# PE broadcast matmul (saves ~12 matmuls + a DVE op per divide)
DIVIDE_BCAST = False  # zero-step partition APs rejected on DVE and DMA paths


def build_nc():
    # xT / xqT / wk / wq / wv arrive pre-swizzled from the host into the
    # exact SBUF tile layouts, so every input DMA is a dense [128, N] copy
    # (full-rate contiguous segments, no strided descriptor storms).
    nc = bacc.Bacc(None, target_bir_lowering=False)
    xT = nc.declare_dram_parameter("xT", [128, 8 * NDC * 512], BF16,
                                   isOutput=False)
    xqT = nc.declare_dram_parameter("xqT", [128, 4 * NDC * 512], BF16,
                                    isOutput=False)
    wk = nc.declare_dram_parameter("wk", [128, NDC * DL], BF16, isOutput=False)
    wq = nc.declare_dram_parameter("wq", [128, NDC * DL], BF16, isOutput=False)
    wv = nc.declare_dram_parameter("wv", [128, NDC * DL], BF16, isOutput=False)
    bkq = nc.declare_dram_parameter("bkq", [DL, 2], F32, isOutput=False)
    bv = nc.declare_dram_parameter("bv", [DL], F32, isOutput=False)
    wout = nc.declare_dram_parameter("wout", [DL, D], BF16, isOutput=False)
    mask64 = nc.declare_dram_parameter("mask64", [128, 64], BF16, isOutput=False)
    ident = nc.declare_dram_parameter("ident", [128, 128], BF16, isOutput=False)
    out = nc.declare_dram_parameter("out", [SQ, D], BF16, isOutput=True)

    from contextlib import ExitStack

    with tile.TileContext(nc) as tc, ExitStack() as ctx:
        persist = ctx.enter_context(tc.tile_pool(name="persist", bufs=1))
        xtp = ctx.enter_context(tc.tile_pool(name="xtp", bufs=1))
        wp = ctx.enter_context(tc.tile_pool(name="wp", bufs=1))
        o0p = ctx.enter_context(tc.tile_pool(name="o0p", bufs=1))
        pjp = ctx.enter_context(tc.tile_pool(name="pjp", bufs=2, space="PSUM"))
        psp = ctx.enter_context(tc.tile_pool(name="psp", bufs=1, space="PSUM"))
        pop = ctx.enter_context(tc.tile_pool(name="pop", bufs=2, space="PSUM"))
        ep = ctx.enter_context(tc.tile_pool(name="ep", bufs=2))
        rp = ctx.enter_context(tc.tile_pool(name="rp", bufs=2))
        osb = ctx.enter_context(tc.tile_pool(name="osb", bufs=3))

        kT01 = persist.tile([128, S], BF16)         # K^T heads 0,1
        kT2 = persist.tile([128, S], BF16)          # K^T head 2 (both halves)
        qT01 = persist.tile([128, SQ], BF16)        # Q^T heads 0,1
        qT2 = persist.tile([128, SQ], BF16)         # Q^T head 2 (both halves)
        aT01 = persist.tile([128, SQ], BF16)        # attn^T heads 0,1
        aT2 = persist.tile([128, SQ], BF16)         # head 2, dup halves
        vbig = persist.tile([128, NKB * HL * VW], BF16)  # V' blocks [k,195]
        bvb = persist.tile([128, DL], F32)          # bv broadcast over rows
        msk = persist.tile([128, 64], BF16)         # causal triangle r<=2c+p
        identp = persist.tile([128, 128], BF16)     # identity (epilogue)
        ones1 = persist.tile([1, 64], BF16)
        bkq0 = persist.tile([128, 2], F32)
        bkq1 = persist.tile([128, 2], F32)          # [0:64] and [64:128] same
        wo0 = persist.tile([128, D], BF16)
        wo1 = persist.tile([128, D], BF16)  # wout[128:DL] duplicated in
        # both partition halves so two wo1 matmuls can dual-stream the PE

        warm = persist.tile([128, 64], BF16)
        nc.gpsimd.memset(warm, 0.125)
        # only V's softmax-sum columns (col 64 of each head's VW stride)
        # need the 1.0 fill; v_proj overwrites the rest. A strided memset
        # keeps the gpsimd queue free for the bias/mask DMAs the head needs.
        nc.gpsimd.memset(
            vbig.rearrange("p (k h v) -> p k h v", h=HL, v=VW)[:, :, :, HD:VW],
            1.0)
        nc.gpsimd.memset(ones1, 1.0)

        # x^T / xq^T land as 512-column slices holding all 6 contraction
        # chunks: tile cols = kc*512 + j. Weights land as [128, 6*DL].
        xt = [xtp.tile([128, NDC * 512], BF16, name=f"xt{n}") for n in range(8)]
        xq = [xtp.tile([128, NDC * 512], BF16, name=f"xq{t}") for t in range(NQT)]
        wk_t = wp.tile([128, NDC * DL], BF16, name="wk")
        wq_t = wp.tile([128, NDC * DL], BF16, name="wq")
        wv_t = wp.tile([128, NDC * DL], BF16, name="wv")

        # input DMAs: dense [128, N] copies (host pre-swizzled), spread
        # across the 3 DMA-capable queues, need-ordered within each queue.
        # xt[0]/xq[0] split by contraction chunk so the head's first
        # matmuls start as soon as their chunk lands.
        TW = NDC * 512  # 3072 cols per x tile

        def dma_x(xts, n, eng, c0=0, c1=NDC):
            eng.dma_start(out=xts[n][:, c0 * 512:c1 * 512],
                          in_=(xT if xts is xt else xqT)[
                              :, n * TW + c0 * 512:n * TW + c1 * 512])

        # Strict need-priority on ONE queue: HBM bandwidth (~360GB/s/core)
        # is shared across queues, so concurrent queues starve the
        # early-critical transfers. Everything big goes on sync in exact
        # consumption order; gpsimd carries only the small tensors.
        nc.sync.dma_start(out=wk_t, in_=wk[:, :])
        dma_x(xt, 0, nc.sync, 0, 2)
        dma_x(xt, 0, nc.sync, 2, 4)
        dma_x(xt, 0, nc.sync, 4, 6)
        nc.sync.dma_start(out=wv_t, in_=wv[:, :])
        nc.sync.dma_start(out=wq_t, in_=wq[:, :])
        dma_x(xq, 0, nc.sync, 0, 3)
        dma_x(xq, 0, nc.sync, 3, 6)
        dma_x(xt, 1, nc.sync)
        dma_x(xq, 1, nc.sync)
        dma_x(xt, 2, nc.sync)
        dma_x(xt, 3, nc.sync)
        dma_x(xt, 4, nc.sync)
        dma_x(xq, 2, nc.sync)
        dma_x(xq, 3, nc.sync)
        dma_x(xt, 5, nc.sync)
        dma_x(xt, 6, nc.sync)
        dma_x(xt, 7, nc.sync)

        # small tensors on the gpsimd queue
        nc.gpsimd.dma_start(out=bkq0, in_=bkq[0:128, :])
        nc.gpsimd.dma_start(out=bkq1[0:64, :], in_=bkq[128:DL, :])
        nc.gpsimd.dma_start(out=bkq1[64:128, :], in_=bkq[128:DL, :])
        nc.gpsimd.dma_start(out=bvb, in_=bv[:].partition_broadcast(128))
        nc.gpsimd.dma_start(out=msk, in_=mask64[:, :])
        nc.gpsimd.dma_start(out=wo0, in_=wout[0:128, :])
        nc.gpsimd.dma_start(out=wo1[0:64, :], in_=wout[128:DL, :])
        nc.gpsimd.dma_start(out=wo1[64:128, :], in_=wout[128:DL, :])
        nc.gpsimd.dma_start(out=identp, in_=ident[:, :])

        # PE preheat: tiny matmuls during the initial DMA wait so the HAM
        # clock-gate's busy window starts counting before the real head
        # projections begin (flip to 8/8 needs ~3.4us of sustained PE
        # activity).
        pwarm = pjp.tile([128, 512], F32, name="pwarm", tag="pj")
        for _ in range(12):
            nc.tensor.matmul(pwarm[0:64, 0:64], lhsT=warm, rhs=warm,
                             start=True, stop=True, skip_group_check=True)

        def kq_proj(dst01, dst2, w_t, rhs, bc, n, m, head=False):
            # dst[m-rows, cols n*512..] = W^T x^T + b  for one m-pass.
            # head=True: rotate through the (idle) score PSUM slots and
            # evacuate on the (idle) scalar engine - no single-slot WAR
            # stall, no DVE backlog.
            nsl = slice(n * 512, (n + 1) * 512)
            mw = 128 if m == 0 else 64
            msl = slice(0, 128) if m == 0 else slice(128, DL)
            if head:
                ps = psp.tile([128, 1024], F32, name="ps", tag="ps",
                              bufs=2)[:, 0:512]
            else:
                ps = pjp.tile([128, 512], F32, name="pj", tag="pj")
            for kc in range(NDC):
                nc.tensor.matmul(
                    ps[:mw, :],
                    lhsT=w_t[:, kc * DL:(kc + 1) * DL][:, msl],
                    rhs=rhs[:, kc * 512:(kc + 1) * 512],
                    start=(kc == 0), stop=(kc == NDC - 1),
                )
            ident = mybir.ActivationFunctionType.Identity

            def evac(dst, src, bias):
                if head:
                    nc.scalar.activation(out=dst, in_=src, func=ident,
                                         bias=bias)
                else:
                    nc.vector.tensor_scalar_add(out=dst, in0=src,
                                                scalar1=bias)

            if m == 0:
                evac(dst01[:, nsl], ps, bkq0[:, bc:bc + 1])
            else:  # head 2: write both partition halves (dual-tile scores)
                evac(dst2[0:64, nsl], ps[:64, :], bkq1[0:64, bc:bc + 1])
                evac(dst2[64:128, nsl], ps[:64, :], bkq1[64:128, bc:bc + 1])

        kqp_state = {}

        def kq1_pair(specs, head=False, part=None):
            # Two m=1 (64-row) projections col-tiled side by side: chain a
            # writes psum partitions 0:64 (array col groups 0-1), chain b
            # partitions 64:128 (groups 2-3); the PE streams both
            # concurrently, halving the m=1 projection time. part=0/1
            # optionally spreads the chunks over two filler pumps.
            key = tuple(s[4] for s in specs)
            if part == 1:
                ps = kqp_state.pop(key)
            elif head:
                ps = psp.tile([128, 1024], F32, name="ps", tag="ps",
                              bufs=2)[:, 0:512]
            else:
                ps = pjp.tile([128, 512], F32, name="pj", tag="pj")
            if part == 0:
                kqp_state[key] = ps
            chunks = (range(0, 3) if part == 0
                      else range(3, NDC) if part == 1 else range(NDC))
            for kc in chunks:
                for half, (dst2, w_t, rhs, bc, n) in zip((0, 64), specs):
                    nc.tensor.matmul(
                        ps[half:half + 64, :],
                        lhsT=w_t[:, kc * DL:(kc + 1) * DL][:, 128:DL],
                        rhs=rhs[:, kc * 512:(kc + 1) * 512],
                        start=(kc == 0), stop=(kc == NDC - 1),
                        skip_group_check=True,
                    )
            if part == 0:
                return
            identf = mybir.ActivationFunctionType.Identity
            for half, (dst2, w_t, rhs, bc, n) in zip((0, 64), specs):
                nsl = slice(n * 512, (n + 1) * 512)
                for dhalf in (0, 64):
                    src = ps[half:half + 64, :]
                    dst = dst2[dhalf:dhalf + 64, nsl]
                    bias = bkq1[dhalf:dhalf + 64, bc:bc + 1]
                    if head:
                        nc.scalar.activation(out=dst, in_=src, func=identf,
                                             bias=bias)
                    else:
                        nc.vector.tensor_scalar_add(out=dst, in0=src,
                                                    scalar1=bias)

        def v_proj(kb, head=False):
            if head:
                pv = psp.tile([128, 1024], F32, name="ps", tag="ps",
                              bufs=2)[:, 0:512]
            else:
                pv = pjp.tile([128, 512], F32, name="pj", tag="pj")
            n, j = kb // 4, (kb % 4) * 128
            for kc in range(NDC):
                nc.tensor.matmul(
                    pv[:, :DL], lhsT=xt[n][:, kc * 512 + j:kc * 512 + j + 128],
                    rhs=wv_t[:, kc * DL:(kc + 1) * DL],
                    start=(kc == 0), stop=(kc == NDC - 1),
                )
            # one strided add writes all 3 heads' V cols (ones col skipped)
            voff = kb * HL * VW
            dstv = vbig[:, voff:voff + HL * VW]
            dstv = dstv.rearrange("p (h vw) -> p h vw", vw=VW)[:, :, 0:HD]
            nc.vector.tensor_add(
                out=dstv,
                in0=pv[:, :DL].rearrange("p (h d) -> p h d", d=HD),
                in1=bvb.rearrange("p (h d) -> p h d", d=HD),
            )

        # per-head score/attn tiles: (lhsT source, rhs source, aT dest)
        kq_src = (
            (kT01, qT01, (0, 64)),     # head 0: always low half
            (kT01, qT01, (64, 128)),   # head 1: always high half
            (kT2, qT2, None),          # head 2: half chosen per matmul
        )
        aT_of = (aT01[0:64], aT01[64:128], aT2[0:64])

        ot_tiles = {}

        def out_proj_half(qt, ncol, pot=None, reg=0, act_copy=False):
            # split at the PSUM-slot reuse boundary so the WAR wait on the
            # previous half's copy never stalls the PE mid-filler
            osl = slice(qt * 128, (qt + 1) * 128)
            if ncol == 0:
                ot_tiles[qt] = osb.tile([128, D], BF16, name="ot", tag="ot")
            ot = ot_tiles[qt]
            cw = 512 if ncol == 0 else 256
            csl = slice(ncol * 512, ncol * 512 + cw)
            final = pot is not None
            if not final:
                pot = pjp.tile([128, 512], F32, name="pj", tag="pj")
            psl = slice(reg * 512, reg * 512 + cw)
            nc.tensor.matmul(
                pot[:, psl], lhsT=aT01[:, osl], rhs=wo0[:, csl],
                start=True, stop=False, skip_group_check=True)
            nc.tensor.matmul(
                pot[:, psl], lhsT=aT2[0:64, osl], rhs=wo1[0:64, csl],
                start=False, stop=True, skip_group_check=True)
            if act_copy:  # tail: split evacuations across DVE and ACT
                nc.scalar.activation(out=ot[:, csl], in_=pot[:, psl],
                                     func=mybir.ActivationFunctionType.Copy)
            else:
                nc.vector.tensor_copy(out=ot[:, csl], in_=pot[:, psl])
            if final:
                nc.gpsimd.dma_start(out=out[osl, csl], in_=ot[:, csl])
            elif ncol == 1:
                nc.gpsimd.dma_start(out=out[osl, :], in_=ot)

        def attention(t, fillers, late_fillers=(), late_start=0,
                      tail_hook=None):
            late_fillers = list(late_fillers)
            nseen = [0]

            def pump(k=1):
                for _ in range(k):
                    nseen[0] += 1
                    if fillers:
                        fillers.pop(0)()
                    elif late_fillers and nseen[0] > late_start:
                        late_fillers.pop(0)()
                    elif k == 1 and nseen[0] % 2 == 0:
                        # spare pump: tiny keep-warm matmul so partial PE
                        # idle never accumulates into a HAM MID window
                        # (re-throttle to 1.2 GHz costs ~2us per event)
                        nc.tensor.matmul(pwarm[0:64, 0:64], lhsT=warm,
                                         rhs=warm, start=True, stop=True,
                                         skip_group_check=True)

            qoff = t * QTW
            last_kb = 8 * t + BAND_PACKS[-1][-1]

            # entry = (half, head, kb, psum_off, width, q_start, band)
            # pack = (entries, exp_ranges, heads_finishing)
            packs = []
            # phase A: heads 0+1 paired on alternating PE row tiles
            for kb in range(0, 8 * t):
                packs.append((
                    [(0, 0, kb, 0, 512, 0, False),
                     (64, 1, kb, 512, 512, 0, False)],
                    ((0, 1024),), ()))
            for pr in BAND_PACKS[:-1]:
                ent = []
                offs = [0, 512]
                for b in pr:
                    w = 512 - 64 * b
                    for h in (0, 1):
                        ent.append((64 * h, h, 8 * t + b, offs[h], w,
                                    64 * b, True))
                        offs[h] += w
                packs.append((ent, ((0, 1024),), ()))
            packs.append((
                [(0, 0, 8 * t + 4, 0, 256, 256, True),
                 (64, 1, 8 * t + 4, 512, 256, 256, True)],
                ((0, 256), (512, 768)), (0, 1)))
            # phase B: head 2 alternating its two duplicated halves
            for kb in range(0, 8 * t, 2):
                packs.append((
                    [(0, 2, kb, 0, 512, 0, False),
                     (64, 2, kb + 1, 512, 512, 0, False)],
                    ((0, 1024),), ()))
            for i, prs in enumerate((((0,), (1, 7)), ((2, 6), (3, 5)))):
                ent = []
                for j, pr in enumerate(prs):
                    off = 512 * j
                    for b in pr:
                        w = 512 - 64 * b
                        ent.append((64 * ((2 * i + j) % 2), 2, 8 * t + b,
                                    off, w, 64 * b, True))
                        off += w
                packs.append((ent, ((0, 1024),), ()))
            packs.append((
                [(0, 2, 8 * t + 4, 0, 256, 256, True)],
                ((0, 256),), (2,)))

            po_of = {}

            def emit_pv(pack, eT):
                entries, _, fin = pack
                for (_half, h, kb, off, w, qs, _band) in entries:
                    if h not in po_of:
                        # lazy: the slot's previous reader (divide of the
                        # evicted head) must already be emitted for the WAR
                        po_of[h] = pop.tile([VW, 512], F32, name="po",
                                            tag="po")
                    voff = kb * HL * VW + h * VW
                    nc.tensor.matmul(
                        po_of[h][0:VW, qs:qs + w],
                        lhsT=vbig[:, voff:voff + VW],
                        rhs=eT[:, off:off + w],
                        start=(kb == 0), stop=(kb == last_kb),
                        skip_group_check=True,
                    )
                if tail_hook is not None and 2 in fin:
                    # last pack of the last tile: emit PE work that does
                    # not depend on the final divide, so it overlaps the
                    # divide's DVE chain instead of queuing behind the
                    # broadcast matmul
                    tail_hook()
                for h in fin:
                    divide(h)

            def divide(h):
                # divide by the softmax sum (row HD of po). The 1->64
                # partition broadcast of the reciprocal goes over a gpsimd
                # DMA (off the PE); the final tile-3 head-2 divide keeps
                # the PE broadcast matmul since it sits on the critical
                # tail and the PE is idle there anyway.
                po = po_of[h]
                if DIVIDE_BCAST and not (t == 3 and h == 2):
                    rec1 = rp.tile([1, 512], F32, name="rec1", tag="rec1")
                    nc.vector.reciprocal_approx_fast(out=rec1,
                                                     in_=po[HD:VW, :])
                    recb = rp.tile([64, 512], F32, name="recb", tag="recb")
                    nc.gpsimd.dma_start(out=recb,
                                        in_=rec1.partition_broadcast(64))
                else:
                    sums = rp.tile([1, 512], BF16, name="sums", tag="sums")
                    nc.vector.tensor_copy(out=sums, in_=po[HD:VW, :])
                    pb = pjp.tile([128, 512], F32, name="pj", tag="pj")
                    nc.tensor.matmul(pb[0:64, :], lhsT=ones1, rhs=sums,
                                     start=True, stop=True)
                    recb = rp.tile([64, 512], F32, name="recb", tag="recb")
                    nc.vector.reciprocal_approx_fast(out=recb,
                                                     in_=pb[0:64, :])
                nc.vector.tensor_mul(
                    out=aT_of[h][:, qoff:qoff + QTW], in0=po[0:HD, :],
                    in1=recb)
                if h == 2:  # mirror into the upper half for dual-stream
                    nc.vector.tensor_copy(
                        out=aT2[64:128, qoff:qoff + QTW],
                        in_=aT2[0:64, qoff:qoff + QTW])

            pend = None  # (pack, eT) whose PV is not yet emitted
            for pack in packs:
                entries, exp_ranges, _fin = pack
                ps = psp.tile([128, 1024], F32, name="ps", tag="ps", bufs=2)
                for (half, h, kb, off, w, qs, _band) in entries:
                    kT_h, qT_h, fixed = kq_src[h]
                    if fixed is not None:
                        hsl = slice(fixed[0], fixed[1])
                    else:
                        hsl = slice(half, half + 64)
                    nc.tensor.matmul(
                        ps[:, off:off + w],
                        lhsT=kT_h[hsl, kb * 128:(kb + 1) * 128],
                        rhs=qT_h[hsl, qoff + qs:qoff + QTW],
                        start=True, stop=True,
                    )
                eT = ep.tile([128, 1024], BF16, name="eT", tag="eT", bufs=3)
                for (r0, r1) in exp_ranges:
                    nc.scalar.activation(
                        out=eT[:, r0:r1], in_=ps[:, r0:r1],
                        func=mybir.ActivationFunctionType.Exp, scale=SCALE)
                for (_half, _h, kb, off, w, qs, band) in entries:
                    if band:  # zero the 64 partial cols of the triangle
                        nc.vector.tensor_mul(
                            out=eT[:, off:off + 64],
                            in0=eT[:, off:off + 64], in1=msk)
                pump(1)
                if pend is not None:
                    emit_pv(*pend)
                pend = (pack, eT)
            emit_pv(*pend)
            pump(len(fillers) + len(late_fillers))

        # wo0-part of the final 4 row-blocks' out-projection: runs as late
        # fillers inside tile 3's phase B (aT01 for t=3 is ready once heads
        # 0/1 finish at the end of phase A); wo1 + combine run in the
        # epilogue once aT2 lands.
        ot0 = {qt: o0p.tile([128, D], BF16, name=f"ot0_{qt}")
               for qt in (12, 13, 14, 15)}

        def out_proj_pair(qta, qtb, ncol):
            # two row-blocks' out-projection halves: the wo0 parts run
            # back-to-back, then the two K=64 wo1 parts dual-stream via
            # aT2's duplicated partition halves.
            cw = 512 if ncol == 0 else 256
            csl = slice(ncol * 512, ncol * 512 + cw)
            pots = []
            for qt in (qta, qtb):
                osl = slice(qt * 128, (qt + 1) * 128)
                if ncol == 0:
                    ot_tiles[qt] = osb.tile([128, D], BF16, name="ot",
                                            tag="ot")
                pot = pjp.tile([128, 512], F32, name="pj", tag="pj")
                nc.tensor.matmul(
                    pot[:, :cw], lhsT=aT01[:, osl], rhs=wo0[:, csl],
                    start=True, stop=False, skip_group_check=True)
                pots.append(pot)
            for i, qt in enumerate((qta, qtb)):
                osl = slice(qt * 128, (qt + 1) * 128)
                hsl = slice(64 * i, 64 * i + 64)
                nc.tensor.matmul(
                    pots[i][:, :cw], lhsT=aT2[hsl, osl], rhs=wo1[hsl, csl],
                    start=False, stop=True, skip_group_check=True)
            for i, qt in enumerate((qta, qtb)):
                osl = slice(qt * 128, (qt + 1) * 128)
                ot = ot_tiles[qt]
                nc.vector.tensor_copy(out=ot[:, csl], in_=pots[i][:, :cw])
                if ncol == 1:
                    nc.gpsimd.dma_start(out=out[osl, :], in_=ot)

        kqh_state = {}

        def kq_half(key, dst01, dst2, w_t, rhs, bc, n, m, part):
            # half of a 6-chunk projection: spreads one filler across two
            # pump slots so filler-heavy packs don't outrun the exp pace
            if part == 0:
                ps = pjp.tile([128, 512], F32, name="pj", tag="pj")
                kqh_state[key] = ps
            else:
                ps = kqh_state.pop(key)
            mw = 128 if m == 0 else 64
            msl = slice(0, 128) if m == 0 else slice(128, DL)
            for kc in (range(0, 3) if part == 0 else range(3, NDC)):
                nc.tensor.matmul(
                    ps[:mw, :],
                    lhsT=w_t[:, kc * DL:(kc + 1) * DL][:, msl],
                    rhs=rhs[:, kc * 512:(kc + 1) * 512],
                    start=(kc == 0), stop=(kc == NDC - 1),
                )
            if part == 1:
                nsl = slice(n * 512, (n + 1) * 512)
                nc.vector.tensor_scalar_add(out=dst01[:, nsl], in0=ps,
                                            scalar1=bkq0[:, bc:bc + 1])

        def wo0_half(qt, ncol):
            osl = slice(qt * 128, (qt + 1) * 128)
            cw = 512 if ncol == 0 else 256
            csl = slice(ncol * 512, ncol * 512 + cw)
            pw = pjp.tile([128, 512], F32, name="pj", tag="pj")
            nc.tensor.matmul(pw[:, :cw], lhsT=aT01[:, osl], rhs=wo0[:, csl],
                             start=True, stop=True, skip_group_check=True)
            nc.vector.tensor_copy(out=ot0[qt][:, csl], in_=pw[:, :cw])

        # ---- schedule: minimal head, then q-tiles t=0..3 with fillers ----
        def K(n, m):
            return lambda: kq_proj(kT01, kT2, wk_t, xt[n], 0, n, m)

        def Q(t, m):
            return lambda: kq_proj(qT01, qT2, wq_t, xq[t], 1, t, m)

        def V(kb):
            return lambda: v_proj(kb)

        def O(qt, ncol):
            return lambda: out_proj_half(qt, ncol)

        def W0(qt, ncol):
            return lambda: wo0_half(qt, ncol)

        def O2(qta, qtb, ncol):
            return lambda: out_proj_pair(qta, qtb, ncol)

        def KH(n, part):
            return lambda: kq_half(("k", n), kT01, kT2, wk_t, xt[n], 0,
                                   n, 0, part)

        def QH(t, part):
            return lambda: kq_half(("q", t), qT01, qT2, wq_t, xq[t], 1,
                                   t, 0, part)

        def KP1(na, nb, part=None):
            return lambda: kq1_pair([(kT2, wk_t, xt[na], 0, na),
                                     (kT2, wk_t, xt[nb], 0, nb)], part=part)

        def QP1(ta, tb):
            return lambda: kq1_pair([(qT2, wq_t, xq[ta], 1, ta),
                                     (qT2, wq_t, xq[tb], 1, tb)])

        # head: only what attention(0)'s phase A needs, DMA-ordered.
        # m=1 halves run as col-tiled pairs (same wall time as one); the
        # q-side m=1 pair rides tile-0's fillers (deadline: its phase B),
        # so the head never waits on the xq1 transfer.
        def hwarm(k=4):
            # bridge the head's DMA-wait pockets with dummy matmuls so the
            # HAM activity window keeps counting and the attention stream
            # enters already at 2.4 GHz
            for _ in range(k):
                nc.tensor.matmul(pwarm[0:64, 0:64], lhsT=warm, rhs=warm,
                                 start=True, stop=True,
                                 skip_group_check=True)

        kq_proj(kT01, kT2, wk_t, xt[0], 0, 0, 0, head=True)
        hwarm()
        v_proj(0, head=True)
        v_proj(1, head=True)
        hwarm(10)
        kq_proj(qT01, qT2, wq_t, xq[0], 1, 0, 0, head=True)
        hwarm(10)
        kq_proj(kT01, kT2, wk_t, xt[1], 0, 1, 0, head=True)
        kq1_pair([(kT2, wk_t, xt[0], 0, 0), (kT2, wk_t, xt[1], 0, 1)],
                 head=True)
        v_proj(7, head=True)

        # V fillers ordered by the band-block usage order of the next tile
        def Vband(t):
            return [V(8 * t + b) for b in (0, 1, 7, 2, 6, 3, 5, 4)]

        # Fillers sized to each tile's pack count (12t+8 pumps) so no tile
        # ends with a serialized dump; each tile's prerequisites (K/Q/V of
        # the NEXT tile) sit early enough in the list to land just in time.
        FILL = {
            0: [V(2), V(6), QP1(0, 1), V(3), V(5), V(4), Q(1, 0), K(2, 0)],
            1: [KH(3, 0), KH(3, 1), KP1(2, 3)] + Vband(1) +
               [KH(4, 0), KH(4, 1), KH(5, 0), KH(5, 1), KP1(4, 5),
                QH(2, 0), QH(2, 1), QP1(2, 3)],
            2: Vband(2) + [KH(6, 0), KH(6, 1), KH(7, 0), KH(7, 1),
                           KP1(6, 7, 0), KP1(6, 7, 1), QH(3, 0), QH(3, 1)],
            3: Vband(3) + [O2(0, 1, 0), O2(0, 1, 1), O2(2, 3, 0),
                           O2(2, 3, 1), O2(4, 5, 0), O2(4, 5, 1),
                           O2(6, 7, 0), O2(6, 7, 1), O2(8, 9, 0),
                           O2(8, 9, 1), O2(10, 11, 0), O2(10, 11, 1)],
        }
        LATE3 = [W0(12, 0), W0(12, 1), W0(13, 0), W0(13, 1),
                 W0(14, 0), W0(14, 1), W0(15, 0), W0(15, 1)]
        EPAIRS = (((12, 0), (13, 0)), ((14, 0), (15, 0)),
                  ((12, 1), (13, 1)), ((14, 1), (15, 1)))
        ep_pts = []

        def ep_ident(pair):
            # wo0 re-injection for one epilogue pair; depends only on the
            # staged ot0 tiles and a free score-psum slot
            pt = psp.tile([128, 1024], F32, name="ps", tag="ps", bufs=2)
            for i, (qt, ncol) in enumerate(pair):
                cw = 512 if ncol == 0 else 256
                csl = slice(ncol * 512, ncol * 512 + cw)
                nc.tensor.matmul(
                    pt[:, 512 * i:512 * i + cw], lhsT=identp,
                    rhs=ot0[qt][:, csl], start=True, stop=False,
                    skip_group_check=True)
            ep_pts.append(pt)

        def tail3():
            ep_ident(EPAIRS[0])
            ep_ident(EPAIRS[1])

        for t in range(NQT):
            if t == 3:
                attention(t, FILL[t], late_fillers=LATE3,
                          late_start=8 * t + 6, tail_hook=tail3)
            else:
                attention(t, FILL[t])
        # epilogue: wo1-part of the final 4 row-blocks (needs aT2 of tile
        # 3). The staged wo0-part is re-injected into PSUM via an identity
        # matmul (PE is idle here), the wo1 pairs dual-stream via aT2's
        # duplicated halves, and a cheap 2x CAST copy replaces a DVE add.
        # Keep-warm dummies bridge the divide(2) stall so the epilogue
        # matmuls run at 2.4 GHz.
        for _ in range(4):
            nc.tensor.matmul(pwarm[0:64, 0:64], lhsT=warm, rhs=warm,
                             start=True, stop=True, skip_group_check=True)
        otf_of = {}
        for ip, ((qta, ncola), (qtb, ncolb)) in enumerate(EPAIRS):
            if ip < len(ep_pts):
                pt = ep_pts[ip]  # wo0 already injected inside tile 3
            else:
                ep_ident(EPAIRS[ip])
                pt = ep_pts[ip]
            for i, (qt, ncol) in enumerate(((qta, ncola), (qtb, ncolb))):
                osl = slice(qt * 128, (qt + 1) * 128)
                hsl = slice(64 * i, 64 * i + 64)
                cw = 512 if ncol == 0 else 256
                csl = slice(ncol * 512, ncol * 512 + cw)
                nc.tensor.matmul(
                    pt[:, 512 * i:512 * i + cw], lhsT=aT2[hsl, osl],
                    rhs=wo1[hsl, csl], start=False, stop=True,
                    skip_group_check=True)
            for i, (qt, ncol) in enumerate(((qta, ncola), (qtb, ncolb))):
                osl = slice(qt * 128, (qt + 1) * 128)
                cw = 512 if ncol == 0 else 256
                csl = slice(ncol * 512, ncol * 512 + cw)
                if qt not in otf_of:
                    otf_of[qt] = osb.tile([128, D], BF16, name="ot", tag="ot")
                otf = otf_of[qt]
                # evacuations and output DMAs alternate engines/queues so
                # the two per-pair chains drain concurrently
                if i == 0:
                    nc.scalar.activation(
                        out=otf[:, csl], in_=pt[:, 0:cw],
                        func=mybir.ActivationFunctionType.Copy)
                    nc.sync.dma_start(out=out[osl, csl], in_=otf[:, csl])
                else:
                    nc.vector.tensor_copy(out=otf[:, csl],
                                          in_=pt[:, 512:512 + cw])
                    nc.gpsimd.dma_start(out=out[osl, csl], in_=otf[:, csl])

    nc.finalize()
    return nc


_NC_CACHE = {}


def _get_nc():
    if "nc" not in _NC_CACHE:
        _NC_CACHE["nc"] = build_nc()
    return _NC_CACHE["nc"]


def kernel(x, Wqkv, bqkv, Wout, bout):
    x = np.asarray(x, dtype=np.float32)
    Wqkv = np.asarray(Wqkv, dtype=np.float32)
    bqkv = np.asarray(bqkv, dtype=np.float32)
    Wout = np.asarray(Wout, dtype=np.float32)
    bout = np.asarray(bout, dtype=np.float32)
    B, S_, D_ = x.shape
    assert (B, S_, D_) == (1, S, D)
    nc = _get_nc()

    xT_np = np.ascontiguousarray(x[0].T).astype(NPBF16)          # [768, 4096]

    def swizzle_x(a):
        # [768, ncols] -> [128, ntiles*6*512]: SBUF tile layout (tile-major,
        # then contraction chunk, then 512 cols) so device DMAs are dense.
        ncols = a.shape[1]
        nt = ncols // 512
        return np.ascontiguousarray(
            a.reshape(6, 128, nt, 512).transpose(1, 2, 0, 3).reshape(
                128, nt * 6 * 512))

    def swizzle_w(w):
        # [768, DL] -> [128, 6*DL]
        return np.ascontiguousarray(
            w.reshape(6, 128, DL).transpose(1, 0, 2).reshape(128, 6 * DL))

    xT_sw = swizzle_x(xT_np)
    xq_sw = [swizzle_x(np.ascontiguousarray(xT_np[:, p::2])) for p in (0, 1)]
    in_maps = []
    for c in range(8):
        g, p = c // 2, c % 2
        csl = slice(DL * g, DL * (g + 1))
        rr = np.arange(128, dtype=np.int64)[:, None]
        cc = np.arange(64, dtype=np.int64)[None, :]
        mask = (rr <= 2 * cc + p).astype(NPBF16)
        bk_h = bqkv[D + DL * g:D + DL * (g + 1)].astype(np.float32)
        bq_h = bqkv[csl].astype(np.float32)
        in_maps.append({
            "xT": xT_sw,
            "xqT": xq_sw[p],
            "wk": swizzle_w(Wqkv[:, D + DL * g:D + DL * (g + 1)].astype(NPBF16)),
            "wq": swizzle_w(Wqkv[:, csl].astype(NPBF16)),
            "wv": swizzle_w(Wqkv[:, 2 * D + DL * g:2 * D + DL * (g + 1)].astype(NPBF16)),
            "bkq": np.ascontiguousarray(np.stack([bk_h, bq_h], axis=1)),
            "bv": np.ascontiguousarray(bqkv[2 * D + DL * g:2 * D + DL * (g + 1)]).astype(np.float32),
            "wout": np.ascontiguousarray(Wout[csl, :]).astype(NPBF16),
            "mask64": mask,
            "ident": np.eye(128, dtype=NPBF16),
        })

    trace = bool(int(os.environ.get("ATTN_TRACE", "0")))
    tmpdir = os.environ.get("ATTN_TMPDIR") or None
    res = run_bass_kernel_spmd(nc, in_maps, core_ids=list(range(8)), trace=trace,
                               tmpdir=tmpdir)
    if trace:
        _NC_CACHE["last_result"] = res

    out_full = np.zeros((S, D), np.float32)
    for p in range(2):
        acc = np.zeros((SQ, D), np.float32)
        for g in range(4):
            acc += res.results[2 * g + p]["out"].astype(np.float32)
        out_full[p::2] = acc
    out_full += bout.astype(np.float32)[None, :]
    return out_full[None].astype(np.float32)

